# revision 1
# baseline (speedup 1.0000x reference)
"""Causal multi-head attention (B=4, T=2048, D=2048, H=16) on 8 Trainium2
NeuronCores via Bass/Tile, SPMD with zero collectives.

Sharding: each batch b is split over two cores by query rows using a
zigzag quarter split (core 2b: quarters Q1+Q4, core 2b+1: quarters Q2+Q3),
which balances the causal-attention triangle. Every core computes K/V
projections for its batch's full sequence (cheap redundancy that keeps the
SPMD program uniform across cores); causal masking is supplied as per-core
input data over a uniform tile pattern, so all 8 cores run the same
instruction stream.

Per-core pipeline (all matmuls in float32r — full PE rate, ~1e-4 rel err):
  0. PE-transpose x -> xT (SBUF slabs, one T/2 half at a time)
  1. K^T = Wk^T xT, Q^T = Wq^T xT (transposed layouts), V = x Wv (natural)
  2. per head: S^T tiles = K^T_chunk^T Q^T, exp on ACT (no max subtraction:
     scores are O(1) by construction), causal/pad masking by DVE multiply,
     A^T accumulated on PE with V as stationary operand, softmax denominators
     via ones-vector matmuls, normalization fused into the PSUM evacuation
  3. O rows = A^T^T Wo + bo
Outputs are the core's own (permuted) query rows; the host scatters them
back into the full [B, T, D] tensor.
"""
import numpy as np

import concourse.bacc as bacc
import concourse.mybir as mybir
from concourse.tile import TileContext
from concourse.bass_utils import run_bass_kernel_spmd

F32 = mybir.dt.float32
F32R = mybir.dt.float32r
EXP = mybir.ActivationFunctionType.Exp
MULT = mybir.AluOpType.mult

PROD_CFG = dict(B=4, T=2048, D=2048, H=16)
PIPELINE = True


def _derived(cfg):
    B, T, D, H = cfg["B"], cfg["T"], cfg["D"], cfg["H"]
    d = dict(cfg)
    d.update(
        QW=T // 4,            # quarter width (query-row shard unit)
        OWN=T // 2,           # own query rows per core
        T2=T // 2,            # xT slab half width
        DK=D // 128,          # contraction chunks
        q=T // 4 // 128,      # 128-row j-tiles per quarter
        NCH=min(512, T // 2),  # moving-N chunk for projections
        ND=min(512, D),       # phase-4 output-column slab width
        DH=128,
        N_CORES=2 * B,
    )
    return d


def _r(ap):
    return ap.bitcast(F32R)


def build_nc(cfg):
    c = _derived(cfg)
    T, D, H = c["T"], c["D"], c["H"]
    QW, OWN, T2, DK, q = c["QW"], c["OWN"], c["T2"], c["DK"], c["q"]
    NCH, ND = c["NCH"], c["ND"]
    SCALE = float(c["DH"] ** -0.5)

    nc = bacc.Bacc(
        "TRN2", target_bir_lowering=False, debug=False, num_devices=c["N_CORES"]
    )
    x = nc.dram_tensor("x", [T, D], F32R, kind="ExternalInput").ap()
    wq = nc.dram_tensor("wq", [D, D], F32R, kind="ExternalInput").ap()
    wk = nc.dram_tensor("wk", [D, D], F32R, kind="ExternalInput").ap()
    wv = nc.dram_tensor("wv", [D, D], F32R, kind="ExternalInput").ap()
    wo = nc.dram_tensor("wo", [D, D], F32R, kind="ExternalInput").ap()
    bq = nc.dram_tensor("bq", [D], F32, kind="ExternalInput").ap()
    bk = nc.dram_tensor("bk", [D], F32, kind="ExternalInput").ap()
    bv = nc.dram_tensor("bv", [D], F32, kind="ExternalInput").ap()
    bo = nc.dram_tensor("bo", [D], F32, kind="ExternalInput").ap()
    mask = nc.dram_tensor("mask", [128, 4 * q * QW], F32R, kind="ExternalInput").ap()
    ident_in = nc.dram_tensor("ident", [128, 128], F32R, kind="ExternalInput").ap()
    ones_c_in = nc.dram_tensor("ones_c", [128, 1], F32R, kind="ExternalInput").ap()
    ones_r_in = nc.dram_tensor("ones_r", [1, 128], F32R, kind="ExternalInput").ap()
    o = nc.dram_tensor("o", [OWN, D], F32, kind="ExternalOutput").ap()

    kt_d = nc.dram_tensor("kt_scratch", [D, T], F32R).ap()
    qt_d = nc.dram_tensor("qt_scratch", [D, OWN], F32R).ap()
    v_d = nc.dram_tensor("v_scratch", [T, D], F32R).ap()

    # uniform causal j-tile windows (see module docstring)
    LWIN = list(range(q)) + list(range(2 * q, 3 * q))          # L+H valid
    HONLY = list(range(q, 2 * q)) + list(range(3 * q, 4 * q))  # H valid only

    with TileContext(nc) as tc:
        with (
            tc.tile_pool(name="const", bufs=1) as pconst,
        ):
            ident = pconst.tile([128, 128], F32R, tag="ident")
            nc.sync.dma_start(out=ident[:], in_=ident_in[:])
            ones_col = pconst.tile([128, 1], F32R, tag="ones_col")
            nc.sync.dma_start(out=ones_col[:], in_=ones_c_in[:])
            ones_row = pconst.tile([1, 128], F32R, tag="ones_row")
            nc.sync.dma_start(out=ones_row[:], in_=ones_r_in[:])
            bk_sb = pconst.tile([128, DK], F32, tag="bk")
            nc.sync.dma_start(out=bk_sb[:], in_=bk.rearrange("(m p) -> p m", p=128))
            bq_sb = pconst.tile([128, DK], F32, tag="bq")
            nc.sync.dma_start(out=bq_sb[:], in_=bq.rearrange("(m p) -> p m", p=128))
            bv_sb = pconst.tile([1, D], F32R, tag="bv")
            nc.sync.dma_start(out=bv_sb[:], in_=bv[None, :].bitcast(F32R))
            bo_sb = pconst.tile([1, D], F32R, tag="bo")
            nc.sync.dma_start(out=bo_sb[:], in_=bo[None, :].bitcast(F32R))

            # ---------------- phase 0+1: xT, K^T, Q^T, V ----------------
            with (
                tc.tile_pool(name="slab", bufs=1) as pslab,
                tc.tile_pool(name="p1x", bufs=2) as p1x,
                tc.tile_pool(name="p1w", bufs=2) as p1w,
                tc.tile_pool(name="p1wv", bufs=2) as p1wv,
                tc.tile_pool(name="p1st", bufs=3) as p1st,
                tc.tile_pool(name="ps_tr", bufs=2, space="PSUM") as ps_tr,
                tc.tile_pool(name="ps_kq", bufs=2, space="PSUM") as ps_kq,
                tc.tile_pool(name="ps_v", bufs=2, space="PSUM") as ps_v,
            ):
                for hf in range(2):
                    slab = pslab.tile([128, DK * T2], F32R, tag="slab")
                    slab3 = slab[:].rearrange("p (k t) -> p k t", k=DK)
                    # transpose x rows [hf*T2, (hf+1)*T2) into slab
                    for tcn in range(T2 // 128):
                        xst = p1x.tile([128, D], F32R, tag="xst")
                        nc.sync.dma_start(
                            out=xst[:],
                            in_=x[hf * T2 + tcn * 128: hf * T2 + (tcn + 1) * 128, :],
                        )
                        for kb in range(0, DK, 4):
                            nb = min(4, DK - kb)
                            ps = ps_tr.tile([128, 512], F32R, tag="pstr")
                            for i in range(nb):
                                nc.tensor.transpose(
                                    ps[:, i * 128:(i + 1) * 128],
                                    xst[:, (kb + i) * 128:(kb + i + 1) * 128],
                                    ident[:],
                                )
                            nc.vector.tensor_copy(
                                slab3[:, kb:kb + nb, tcn * 128:(tcn + 1) * 128],
                                ps[:, : nb * 128].rearrange(
                                    "p (a b) -> p a b", a=nb
                                ),
                            )
                    # K^T (and Q^T on half 0) projections
                    projs = [(wk, bk_sb, kt_d, True)]
                    if hf == 0:
                        projs.append((wq, bq_sb, qt_d, False))
                    for w_in, b_sb, out_d, is_k in projs:
                        for m in range(DK):
                            wm = p1w.tile([128, DK * 128], F32R, tag="wm")
                            nc.sync.dma_start(
                                out=wm[:],
                                in_=w_in.rearrange("(k p) n -> p k n", p=128)[
                                    :, :, m * 128:(m + 1) * 128
                                ],
                            )
                            for jt in range(T2 // NCH):
                                ps = ps_kq.tile([128, NCH], F32, tag="pskq")
                                for k in range(DK):
                                    nc.tensor.matmul(
                                        ps[:],
                                        _r(wm[:, k * 128:(k + 1) * 128]),
                                        _r(slab[:, k * T2 + jt * NCH:
                                                k * T2 + (jt + 1) * NCH]),
                                        start=(k == 0),
                                        stop=(k == DK - 1),
                                    )
                                st = p1st.tile([128, NCH], F32R, tag="kqst")
                                nc.vector.tensor_scalar_add(
                                    st[:], ps[:], b_sb[:, m:m + 1]
                                )
                                col0 = (hf * T2 if is_k else 0) + jt * NCH
                                nc.sync.dma_start(
                                    out=out_d[m * 128:(m + 1) * 128,
                                              col0:col0 + NCH],
                                    in_=st[:],
                                )
                    # V projection (natural layout), n-chunks of 512
                    for nb_ in range(D // min(512, D)):
                        NV = min(512, D)
                        wvn = p1wv.tile([128, DK * NV], F32R, tag="wvn")
                        nc.sync.dma_start(
                            out=wvn[:],
                            in_=wv.rearrange("(k p) n -> p k n", p=128)[
                                :, :, nb_ * NV:(nb_ + 1) * NV
                            ],
                        )
                        for tcn in range(T2 // 128):
                            ps = ps_v.tile([128, NV], F32, tag="psv")
                            for k in range(DK):
                                nc.tensor.matmul(
                                    ps[:],
                                    _r(slab[:, k * T2 + tcn * 128:
                                            k * T2 + (tcn + 1) * 128]),
                                    _r(wvn[:, k * NV:(k + 1) * NV]),
                                    start=(k == 0),
                                    stop=False,
                                )
                            nc.tensor.matmul(
                                ps[:],
                                _r(ones_row[:]),
                                _r(bv_sb[:, nb_ * NV:(nb_ + 1) * NV]),
                                start=False,
                                stop=True,
                            )
                            st = p1st.tile([128, NV], F32R, tag="vst")
                            nc.scalar.copy(st[:], ps[:])
                            nc.sync.dma_start(
                                out=v_d[hf * T2 + tcn * 128:
                                        hf * T2 + (tcn + 1) * 128,
                                        nb_ * NV:(nb_ + 1) * NV],
                                in_=st[:],
                            )

            # ---------------- phase 2+3: attention per head ----------------
            with tc.tile_pool(name="aslab", bufs=1) as paslab:
              at_sb = paslab.tile([128, H * OWN], F32R, tag="aslab")
              with (
                tc.tile_pool(name="pmask", bufs=1) as pmask,
                tc.tile_pool(name="ph", bufs=2) as ph,
                tc.tile_pool(name="ppt", bufs=3) as ppt,
                tc.tile_pool(name="psm", bufs=2) as psm,
                tc.tile_pool(name="ps_s", bufs=2, space="PSUM") as ps_s,
                tc.tile_pool(name="ps_a", bufs=1, space="PSUM") as ps_a,
                tc.tile_pool(name="ps_l", bufs=1, space="PSUM") as ps_l,
            ):
                mask_sb = pmask.tile([128, 4 * q * QW], F32R, tag="mask")
                nc.sync.dma_start(out=mask_sb[:], in_=mask[:])
                NS = min(512, OWN)
                for h in range(H):
                    kt_h = ph.tile([128, T], F32R, tag="kth")
                    nc.sync.dma_start(
                        out=kt_h[:], in_=kt_d[h * 128:(h + 1) * 128, :]
                    )
                    qt_h = ph.tile([128, OWN], F32R, tag="qth")
                    nc.sync.dma_start(
                        out=qt_h[:], in_=qt_d[h * 128:(h + 1) * 128, :]
                    )
                    v_h = ph.tile([128, T], F32R, tag="vh")
                    nc.sync.dma_start(
                        out=v_h[:].rearrange("p (jb c) -> p jb c", c=128),
                        in_=v_d.rearrange("(jb p) d -> p jb d", p=128)[
                            :, :, h * 128:(h + 1) * 128
                        ],
                    )
                    psa = ps_a.tile([128, OWN], F32, tag="psa")
                    psl = ps_l.tile([1, OWN], F32, tag="psl")
                    h_own_bank = QW * 4 >= 2048
                    n_tiles = len(LWIN) + len(HONLY)

                    def consume(jb, ptv, full, pos):
                        # AV + denominator matmuls for a tile whose exp/mask
                        # chain was issued one pipeline step earlier.
                        vt = _r(v_h[:, jb * 128:(jb + 1) * 128])
                        first = pos == 0
                        # stop clears the (bank-granular) sim group flag, so in
                        # the shared-bank layout only the final H write stops
                        last_l = (pos == len(LWIN) - 1) and h_own_bank
                        last_h = pos == n_tiles - 1
                        if first and not h_own_bank:
                            nc.tensor.matmul(
                                psa[:, :OWN], vt, _r(ptv[:, :OWN]),
                                start=True, stop=False,
                            )
                            nc.tensor.matmul(
                                psl[:, :OWN], _r(ones_col[:]), _r(ptv[:, :OWN]),
                                start=True, stop=False,
                            )
                            return
                        if full:
                            nc.tensor.matmul(
                                psa[:, :QW], vt, _r(ptv[:, :QW]),
                                start=first, stop=last_l,
                            )
                            nc.tensor.matmul(
                                psa[:, QW:OWN], vt, _r(ptv[:, QW:OWN]),
                                start=first and h_own_bank, stop=last_h,
                            )
                            nc.tensor.matmul(
                                psl[:, :QW], _r(ones_col[:]), _r(ptv[:, :QW]),
                                start=first, stop=last_l,
                            )
                            nc.tensor.matmul(
                                psl[:, QW:OWN], _r(ones_col[:]),
                                _r(ptv[:, QW:OWN]),
                                start=first and h_own_bank, stop=last_h,
                            )
                        else:
                            nc.tensor.matmul(
                                psa[:, QW:OWN], vt, _r(ptv[:, :QW]),
                                start=False, stop=last_h,
                            )
                            nc.tensor.matmul(
                                psl[:, QW:OWN], _r(ones_col[:]),
                                _r(ptv[:, :QW]),
                                start=False, stop=last_h,
                            )

                    # units: full tiles singly; H-only tiles in PAIRS sharing
                    # one PSUM slot, one wide exp and one wide mask-multiply
                    # (halves ACT/DVE instruction overhead in the softmax).
                    units = [("full", (jb,)) for jb in LWIN] + [
                        ("hpair", tuple(HONLY[i:i + 2]))
                        for i in range(0, len(HONLY), 2)
                    ]
                    pos = 0
                    pending = []
                    for kind, jjs in units:
                        pss = ps_s.tile([128, OWN], F32, tag="pss")
                        pt = ppt.tile([128, OWN], F32R, tag="pt")
                        if kind == "full":
                            (jb,) = jjs
                            ns = min(NS, OWN)
                            for sc in range(OWN // ns):
                                nc.tensor.matmul(
                                    pss[:, sc * ns:(sc + 1) * ns],
                                    _r(kt_h[:, jb * 128:(jb + 1) * 128]),
                                    _r(qt_h[:, sc * ns:(sc + 1) * ns]),
                                    start=True, stop=True,
                                )
                            nc.scalar.activation(pt[:], pss[:], EXP, scale=SCALE)
                            mc = LWIN.index(jb) * QW
                            nc.vector.tensor_mul(
                                pt[:, :QW], pt[:, :QW], mask_sb[:, mc:mc + QW]
                            )
                            fresh = [(jb, pt[:], True)]
                        else:
                            for half, jb in enumerate(jjs):
                                nc.tensor.matmul(
                                    pss[:, half * QW:(half + 1) * QW],
                                    _r(kt_h[:, jb * 128:(jb + 1) * 128]),
                                    _r(qt_h[:, QW:OWN]),
                                    start=True, stop=True,
                                )
                            w = len(jjs) * QW
                            nc.scalar.activation(
                                pt[:, :w], pss[:, :w], EXP, scale=SCALE
                            )
                            mc = (2 * q + HONLY.index(jjs[0])) * QW
                            nc.vector.tensor_mul(
                                pt[:, :w], pt[:, :w], mask_sb[:, mc:mc + w]
                            )
                            fresh = [
                                (jb, pt[:, half * QW:(half + 1) * QW], False)
                                for half, jb in enumerate(jjs)
                            ]
                        if not PIPELINE:
                            pending.extend(fresh)
                            fresh = []
                        for jb_, ptv_, full_ in pending:
                            consume(jb_, ptv_, full_, pos)
                            pos += 1
                        pending = fresh
                    for jb_, ptv_, full_ in pending:
                        consume(jb_, ptv_, full_, pos)
                        pos += 1
                    # Evacuate both PSUM accumulators with fast ACT copies so
                    # the next head's matmuls aren't gated on the (slow)
                    # reciprocal / broadcast / normalize chain below.
                    l_raw = psm.tile([1, OWN], F32, tag="lraw")
                    nc.vector.tensor_copy(l_raw[:], psl[:])
                    at_raw = psm.tile([128, OWN], F32, tag="atraw")
                    nc.vector.tensor_copy(at_raw[:], psa[:])
                    l_sb = psm.tile([1, OWN], F32, tag="lsb")
                    nc.vector.reciprocal_approx_fast(l_sb[:], l_raw[:])
                    lb = psm.tile([128, OWN], F32, tag="lb")
                    nc.gpsimd.partition_broadcast(lb[:], l_sb[:], channels=128)
                    nc.vector.tensor_tensor(
                        at_sb[:, h * OWN:(h + 1) * OWN], at_raw[:], lb[:], MULT
                    )

              # ---------------- phase 4: output projection ----------------
              with (
                  tc.tile_pool(name="p4w", bufs=2) as p4w,
                  tc.tile_pool(name="p4st", bufs=2) as p4st,
                  tc.tile_pool(name="ps_o", bufs=2, space="PSUM") as ps_o,
              ):
                  for nh in range(D // ND):
                      won = p4w.tile([128, DK * ND], F32R, tag="won")
                      nc.sync.dma_start(
                          out=won[:],
                          in_=wo.rearrange("(k p) n -> p k n", p=128)[
                              :, :, nh * ND:(nh + 1) * ND
                          ],
                      )
                      for tt in range(OWN // 128):
                          pso = ps_o.tile([128, ND], F32, tag="pso")
                          for k in range(DK):
                              for sc in range(ND // min(512, ND)):
                                  NO = min(512, ND)
                                  nc.tensor.matmul(
                                      pso[:, sc * NO:(sc + 1) * NO],
                                      at_sb[:, k * OWN + tt * 128:
                                            k * OWN + (tt + 1) * 128],
                                      _r(won[:, k * ND + sc * NO:
                                             k * ND + (sc + 1) * NO]),
                                      start=(k == 0),
                                      stop=False,
                                  )
                          for sc in range(ND // min(512, ND)):
                              NO = min(512, ND)
                              nc.tensor.matmul(
                                  pso[:, sc * NO:(sc + 1) * NO],
                                  _r(ones_row[:]),
                                  _r(bo_sb[:, nh * ND + sc * NO:
                                           nh * ND + (sc + 1) * NO]),
                                  start=False,
                                  stop=True,
                              )
                          ost = p4st.tile([128, ND], F32, tag="ost")
                          nc.scalar.copy(ost[:], pso[:])
                          nc.sync.dma_start(
                              out=o[tt * 128:(tt + 1) * 128, nh * ND:(nh + 1) * ND],
                              in_=ost[:],
                          )
    nc.compile()
    return nc


def host_shard(cfg, x_full):
    """Per-core permutations, permuted x, and mask tensors.

    Returns (perms, x_ins, masks): lists indexed by core = 2*b + z.
    """
    c = _derived(cfg)
    B, T, QW, OWN, q = c["B"], c["T"], c["QW"], c["OWN"], c["q"]
    quarters = [np.arange(i * QW, (i + 1) * QW) for i in range(4)]
    LWIN = list(range(q)) + list(range(2 * q, 3 * q))
    HONLY = list(range(q, 2 * q)) + list(range(3 * q, 4 * q))
    perms, x_ins, masks = [], [], []
    for b in range(B):
        for z in range(2):
            if z == 0:
                own = [quarters[0], quarters[3]]
                rest = [quarters[1], quarters[2]]
            else:
                own = [quarters[1], quarters[2]]
                rest = [quarters[0], quarters[3]]
            perm = np.concatenate(own + rest)
            perms.append(perm)
            x_ins.append(np.ascontiguousarray(x_full[b][perm]))
            m = np.empty((128, 4 * q * QW), dtype=np.float32)
            ig_L = perm[:QW]
            ig_H = perm[QW:OWN]
            for t, jb in enumerate(LWIN):
                jg = perm[jb * 128:(jb + 1) * 128]
                m[:, t * QW:(t + 1) * QW] = (
                    jg[:, None] <= ig_L[None, :]
                ).astype(np.float32)
            for t, jb in enumerate(HONLY):
                jg = perm[jb * 128:(jb + 1) * 128]
                m[:, (2 * q + t) * QW:(2 * q + t + 1) * QW] = (
                    jg[:, None] <= ig_H[None, :]
                ).astype(np.float32)
            masks.append(m)
    return perms, x_ins, masks


def run_cores(cfg, nc, inputs, perms, x_ins, masks, trace=False, tmpdir=None):
    c = _derived(cfg)
    n = c["N_CORES"]
    f32 = np.float32
    shared = {
        "wq": np.ascontiguousarray(inputs["Wq"], f32),
        "wk": np.ascontiguousarray(inputs["Wk"], f32),
        "wv": np.ascontiguousarray(inputs["Wv"], f32),
        "wo": np.ascontiguousarray(inputs["Wo"], f32),
        "bq": np.ascontiguousarray(inputs["bq"], f32),
        "bk": np.ascontiguousarray(inputs["bk"], f32),
        "bv": np.ascontiguousarray(inputs["bv"], f32),
        "bo": np.ascontiguousarray(inputs["bo"], f32),
    }
    consts = {
        "ident": np.eye(128, dtype=f32),
        "ones_c": np.ones((128, 1), f32),
        "ones_r": np.ones((1, 128), f32),
    }
    in_maps = [
        {"x": x_ins[i], "mask": masks[i], **consts, **shared} for i in range(n)
    ]
    res = run_bass_kernel_spmd(
        nc, in_maps, list(range(n)), trace=trace, tmpdir=tmpdir
    )
    B, T, D, OWN = c["B"], c["T"], c["D"], c["OWN"]
    out = np.empty((B, T, D), dtype=np.float32)
    for b in range(B):
        for z in range(2):
            core = 2 * b + z
            out[b][perms[core][:OWN]] = res.results[core]["o"]
    return out, res


_NC_CACHE = {}


def kernel(x, Wq, bq, Wk, bk, Wv, bv, Wo, bo):
    cfg = PROD_CFG
    key = tuple(sorted(cfg.items()))
    if key not in _NC_CACHE:
        _NC_CACHE[key] = build_nc(cfg)
    nc = _NC_CACHE[key]
    x = np.asarray(x, np.float32)
    perms, x_ins, masks = host_shard(cfg, x)
    inputs = dict(Wq=Wq, bq=bq, Wk=Wk, bk=bk, Wv=Wv, bv=bv, Wo=Wo, bo=bo)
    out, _ = run_cores(cfg, nc, inputs, perms, x_ins, masks)
    return out



# revision 2
# speedup vs baseline: 1.4300x; 1.4300x over previous
"""Causal multi-head attention (B=4, T=2048, D=2048, H=16) on 8 Trainium2
NeuronCores via Bass/Tile, SPMD with zero collectives.

Sharding: core = (batch b, head-half hg). Each core owns one batch and 8 of
the 16 heads: it projects Q/K/V for its 1024-column slice of Wq/Wk/Wv over
the full sequence, runs causal attention for its 8 heads, and computes the
partial output projection A @ Wo[hg*1024:(hg+1)*1024, :]. The host feeds
x^T per batch (so no on-device transpose) and sums the two partials per
batch (bo is folded into the hg=0 partial on device via a broadcast tile).

Per-core pipeline (all matmuls f32r, moving dim 512 = full PE rate):
  1. K^T = Wk_s^T xT, Q^T = Wq_s^T xT (transposed layouts, bias fused into
     the PSUM evacuation), V = x Wv_s (natural layout, bias via broadcast
     tile) -> DRAM scratch
  2. per head: S^T supertiles (512 queries) = K^T_blk^T Q^T, exp on ACT
     (no max subtraction: scores are O(1)), causal diagonal masking by DVE
     multiply, A^T accumulated on PE with V as stationary, softmax
     denominators by DVE-accumulated exp sums + one ones-vector matmul per
     supertile, normalization on the PSUM evacuation
  3. O partial rows = A^T^T Wo_s (+ bo on hg=0 cores)
"""
import numpy as np

import concourse.bacc as bacc
import concourse.mybir as mybir
from concourse.tile import TileContext
from concourse.bass_utils import run_bass_kernel_spmd

F32 = mybir.dt.float32
F32R = mybir.dt.float32r
EXP = mybir.ActivationFunctionType.Exp
MULT = mybir.AluOpType.mult
ADD = mybir.AluOpType.add

PROD_CFG = dict(B=4, T=2048, D=2048, H=16)


def _derived(cfg):
    B, T, D, H = cfg["B"], cfg["T"], cfg["D"], cfg["H"]
    d = dict(cfg)
    d.update(
        HN=H // 2,            # heads per core
        DHD=(H // 2) * (D // H),  # local head dim total (1024)
        DK=D // 128,          # contraction chunks of x^T
        SS=512,               # query supertile width
        DH=D // H,            # 128
        N_CORES=2 * B,
    )
    return d


def _r(ap):
    return ap.bitcast(F32R)


def build_nc(cfg):
    c = _derived(cfg)
    T, D = c["T"], c["D"]
    HN, DHD, DK, SS = c["HN"], c["DHD"], c["DK"], c["SS"]
    NB = T // 128          # key blocks (16)
    NST = T // SS          # supertiles (4)
    JPS = SS // 128        # key blocks per supertile (4)
    SCALE = float(c["DH"] ** -0.5)

    nc = bacc.Bacc(
        "TRN2", target_bir_lowering=False, debug=False, num_devices=c["N_CORES"]
    )
    xt = nc.dram_tensor("xt", [D, T], F32R, kind="ExternalInput").ap()
    wk = nc.dram_tensor("wk", [D, DHD], F32R, kind="ExternalInput").ap()
    wq = nc.dram_tensor("wq", [D, DHD], F32R, kind="ExternalInput").ap()
    wv = nc.dram_tensor("wv", [D, DHD], F32R, kind="ExternalInput").ap()
    wo = nc.dram_tensor("wo", [DHD, D], F32R, kind="ExternalInput").ap()
    bkq = nc.dram_tensor("bkq", [128, 2 * HN], F32, kind="ExternalInput").ap()
    bvb = nc.dram_tensor("bvb", [128, DHD], F32R, kind="ExternalInput").ap()
    bob = nc.dram_tensor("bob", [128, D], F32R, kind="ExternalInput").ap()
    mask = nc.dram_tensor("mask", [128, JPS * SS], F32R, kind="ExternalInput").ap()
    ones_c_in = nc.dram_tensor("ones_c", [128, 1], F32R, kind="ExternalInput").ap()
    o = nc.dram_tensor("o", [T, D], F32, kind="ExternalOutput").ap()

    kt_d = nc.dram_tensor("kt_scratch", [DHD, T], F32R).ap()
    qt_d = nc.dram_tensor("qt_scratch", [DHD, T], F32R).ap()
    v_d = nc.dram_tensor("v_scratch", [T, DHD], F32R).ap()

    with TileContext(nc) as tc:
        with tc.tile_pool(name="const", bufs=1) as pconst:
            ones_col = pconst.tile([128, 1], F32R, tag="ones_col")
            nc.sync.dma_start(out=ones_col[:], in_=ones_c_in[:])
            bkq_sb = pconst.tile([128, 2 * HN], F32, tag="bkq")
            nc.sync.dma_start(out=bkq_sb[:], in_=bkq[:])
            bv_sb = pconst.tile([128, DHD], F32R, tag="bv")
            nc.sync.dma_start(out=bv_sb[:], in_=bvb[:])

            # ---------------- phase 1: K^T, Q^T, V ----------------
            with (
                tc.tile_pool(name="slab", bufs=1) as pslab,
                tc.tile_pool(name="p1w", bufs=2) as p1w,
                tc.tile_pool(name="p1wv", bufs=1) as p1wv,
                tc.tile_pool(name="p1st", bufs=3) as p1st,
                tc.tile_pool(name="ps_kq", bufs=2, space="PSUM") as ps_kq,
                tc.tile_pool(name="ps_v", bufs=2, space="PSUM") as ps_v,
            ):
                slab = pslab.tile([128, DK * T], F32R, tag="slab")
                slab3 = slab[:].rearrange("p (k t) -> p k t", k=DK)
                xt3 = xt.rearrange("(k p) t -> p k t", p=128)
                # split by k so matmuls start as chunks land
                for k in range(DK):
                    nc.sync.dma_start(out=slab3[:, k, :], in_=xt3[:, k, :])

                # K^T and Q^T projections (transposed layout, 128-row chunks)
                for pi, (w_in, out_d) in enumerate(((wk, kt_d), (wq, qt_d))):
                    w3 = w_in.rearrange("(k p) n -> p k n", p=128)
                    for m in range(HN):
                        wblk = p1w.tile([128, DK * 128], F32R, tag="wblk")
                        wblk3 = wblk[:].rearrange("p (k n) -> p k n", k=DK)
                        nc.sync.dma_start(
                            out=wblk[:], in_=w3[:, :, m * 128:(m + 1) * 128]
                        )
                        for tcn in range(T // 512):
                            ps = ps_kq.tile([128, 512], F32, tag="pskq")
                            for k in range(DK):
                                nc.tensor.matmul(
                                    ps[:],
                                    wblk3[:, k, :],
                                    slab3[:, k, tcn * 512:(tcn + 1) * 512],
                                    start=(k == 0),
                                    stop=(k == DK - 1),
                                )
                            st = p1st.tile([128, 512], F32R, tag="kqst")
                            nc.vector.tensor_scalar_add(
                                st[:], ps[:], bkq_sb[:, pi * HN + m:pi * HN + m + 1]
                            )
                            nc.sync.dma_start(
                                out=out_d[m * 128:(m + 1) * 128,
                                          tcn * 512:(tcn + 1) * 512],
                                in_=st[:],
                            )

                # V projection (natural layout), col halves of 512
                wv3 = wv.rearrange("(k p) n -> p k n", p=128)
                for cc in range(DHD // 512):
                    wvn = p1wv.tile([128, DK * 512], F32R, tag="wvn")
                    wvn3 = wvn[:].rearrange("p (k n) -> p k n", k=DK)
                    nc.sync.dma_start(
                        out=wvn[:], in_=wv3[:, :, cc * 512:(cc + 1) * 512]
                    )
                    for tb in range(NB):
                        ps = ps_v.tile([128, 512], F32, tag="psv")
                        for k in range(DK):
                            nc.tensor.matmul(
                                ps[:],
                                slab3[:, k, tb * 128:(tb + 1) * 128],
                                wvn3[:, k, :],
                                start=(k == 0),
                                stop=(k == DK - 1),
                            )
                        st = p1st.tile([128, 512], F32R, tag="vst")
                        nc.vector.tensor_tensor(
                            st[:], ps[:], bv_sb[:, cc * 512:(cc + 1) * 512], ADD
                        )
                        nc.sync.dma_start(
                            out=v_d[tb * 128:(tb + 1) * 128,
                                    cc * 512:(cc + 1) * 512],
                            in_=st[:],
                        )

            # ---------------- phase 2: attention per head ----------------
            with tc.tile_pool(name="aslab", bufs=1) as paslab:
                at_sb = paslab.tile([128, HN * T], F32R, tag="aslab")
                at3 = at_sb[:].rearrange("p (h t) -> p h t", h=HN)
                with (
                    tc.tile_pool(name="pmask", bufs=1) as pmask,
                    tc.tile_pool(name="ph", bufs=2) as ph,
                    tc.tile_pool(name="pe", bufs=3) as pe_pool,
                    tc.tile_pool(name="peacc", bufs=6) as peacc,
                    tc.tile_pool(name="paraw", bufs=6) as paraw,
                    tc.tile_pool(name="psm", bufs=4) as psm,
                    tc.tile_pool(name="plb", bufs=4) as plb,
                    tc.tile_pool(name="ps_s", bufs=3, space="PSUM") as ps_s,
                    tc.tile_pool(name="ps_a", bufs=2, space="PSUM") as ps_a,
                    tc.tile_pool(name="ps_l", bufs=2, space="PSUM") as ps_l,
                ):
                    mask_sb = pmask.tile([128, JPS * SS], F32R, tag="mask")
                    nc.sync.dma_start(out=mask_sb[:], in_=mask[:])
                    v_dr = v_d.rearrange("(jb p) c -> p jb c", p=128)
                    for h in range(HN):
                        kt_h = ph.tile([128, T], F32R, tag="kth")
                        nc.sync.dma_start(
                            out=kt_h[:], in_=kt_d[h * 128:(h + 1) * 128, :]
                        )
                        qt_h = ph.tile([128, T], F32R, tag="qth")
                        nc.sync.dma_start(
                            out=qt_h[:], in_=qt_d[h * 128:(h + 1) * 128, :]
                        )
                        v_h = ph.tile([128, NB * 128], F32R, tag="vh")
                        v_h3 = v_h[:].rearrange("p (j c) -> p j c", j=NB)
                        nc.sync.dma_start(
                            out=v_h3[:],
                            in_=v_dr[:, :, h * 128:(h + 1) * 128],
                        )

                        def finish_supertile(s, psa, eacc):
                            # denominator + normalization chain; psa is
                            # copied out fast so its bank frees for s+1
                            araw = paraw.tile([128, SS], F32, tag="araw")
                            nc.vector.tensor_copy(araw[:], psa[:])
                            pd = ps_l.tile([1, SS], F32, tag="pd")
                            nc.tensor.matmul(
                                pd[:], ones_col[:], eacc[:],
                                start=True, stop=True,
                            )
                            l_sb = psm.tile([1, SS], F32, tag="lsb")
                            nc.vector.tensor_copy(l_sb[:], pd[:])
                            linv = psm.tile([1, SS], F32, tag="linv")
                            nc.vector.reciprocal_approx_fast(linv[:], l_sb[:])
                            lb = plb.tile([128, SS], F32, tag="lb")
                            nc.gpsimd.partition_broadcast(
                                lb[:], linv[:], channels=128
                            )
                            nc.vector.tensor_tensor(
                                at3[:, h, s * SS:(s + 1) * SS],
                                araw[:], lb[:], MULT,
                            )

                        pending = None
                        for s in range(NST):
                            psa = ps_a.tile([128, SS], F32, tag="psa")
                            eacc = peacc.tile([128, SS], F32R, tag="eacc")
                            nj = JPS * s + JPS
                            for j in range(nj):
                                pss = ps_s.tile([128, SS], F32, tag="pss")
                                nc.tensor.matmul(
                                    pss[:],
                                    kt_h[:, j * 128:(j + 1) * 128],
                                    qt_h[:, s * SS:(s + 1) * SS],
                                    start=True, stop=True,
                                )
                                et = pe_pool.tile([128, SS], F32R, tag="et")
                                nc.scalar.activation(
                                    et[:], pss[:], EXP, scale=SCALE
                                )
                                if j >= JPS * s:
                                    mc = (j - JPS * s) * SS
                                    nc.vector.tensor_mul(
                                        et[:], et[:], mask_sb[:, mc:mc + SS]
                                    )
                                if pending is not None:
                                    pending()
                                vj = v_h3[:, j, :]
                                first, last = (j == 0), (j == nj - 1)

                                def consume(et=et, vj=vj, first=first,
                                            last=last, psa=psa, eacc=eacc,
                                            s=s):
                                    nc.tensor.matmul(
                                        psa[:], vj, et[:],
                                        start=first, stop=last,
                                    )
                                    if first:
                                        nc.vector.tensor_copy(eacc[:], et[:])
                                    else:
                                        nc.vector.tensor_tensor(
                                            eacc[:], eacc[:], et[:], ADD
                                        )
                                    if last:
                                        finish_supertile(s, psa, eacc)

                                pending = consume
                        pending()
                        pending = None

                # ---------------- phase 3: output projection ----------------
                with (
                    tc.tile_pool(name="p3w", bufs=1) as p3w,
                    tc.tile_pool(name="p3b", bufs=1) as p3b,
                    tc.tile_pool(name="p3st", bufs=3) as p3st,
                    tc.tile_pool(name="ps_o", bufs=2, space="PSUM") as ps_o,
                ):
                    bo_sb = p3b.tile([128, D], F32R, tag="bo")
                    nc.sync.dma_start(out=bo_sb[:], in_=bob[:])
                    wo_sb = p3w.tile([128, HN * D], F32R, tag="wo")
                    wo3 = wo_sb[:].rearrange("p (h n) -> p h n", h=HN)
                    wo_r = wo.rearrange("(k p) n -> p k n", p=128)
                    for h in range(HN):
                        nc.sync.dma_start(out=wo3[:, h, :], in_=wo_r[:, h, :])
                    for tb in range(NB):
                        for cc in range(D // 512):
                            pso = ps_o.tile([128, 512], F32, tag="pso")
                            for h in range(HN):
                                nc.tensor.matmul(
                                    pso[:],
                                    at3[:, h, tb * 128:(tb + 1) * 128],
                                    wo3[:, h, cc * 512:(cc + 1) * 512],
                                    start=(h == 0),
                                    stop=(h == HN - 1),
                                )
                            ost = p3st.tile([128, 512], F32, tag="ost")
                            nc.vector.tensor_tensor(
                                ost[:], pso[:],
                                bo_sb[:, cc * 512:(cc + 1) * 512], ADD
                            )
                            nc.sync.dma_start(
                                out=o[tb * 128:(tb + 1) * 128,
                                      cc * 512:(cc + 1) * 512],
                                in_=ost[:],
                            )
    nc.compile()
    return nc


def make_core_inputs(cfg, inputs):
    """Per-core input maps. Core index = 2*b + hg."""
    c = _derived(cfg)
    B, T, D, H = c["B"], c["T"], c["D"], c["H"]
    HN, DHD, SS = c["HN"], c["DHD"], c["SS"]
    JPS = SS // 128
    f32 = np.float32
    x = np.asarray(inputs["x"], f32)
    Wk = np.asarray(inputs["Wk"], f32)
    Wq = np.asarray(inputs["Wq"], f32)
    Wv = np.asarray(inputs["Wv"], f32)
    Wo = np.asarray(inputs["Wo"], f32)
    bk = np.asarray(inputs["bk"], f32)
    bq = np.asarray(inputs["bq"], f32)
    bv = np.asarray(inputs["bv"], f32)
    bo = np.asarray(inputs["bo"], f32)

    p = np.arange(128)[:, None]
    cq = np.arange(SS)[None, :]
    mask = np.empty((128, JPS * SS), dtype=f32)
    for jj in range(JPS):
        mask[:, jj * SS:(jj + 1) * SS] = (jj * 128 + p <= cq).astype(f32)
    ones_c = np.ones((128, 1), f32)

    per_hg = []
    for hg in range(2):
        sl = slice(hg * DHD, (hg + 1) * DHD)
        bkq = np.empty((128, 2 * HN), f32)
        bkq[:, :HN] = bk[sl].reshape(HN, 128).T
        bkq[:, HN:] = bq[sl].reshape(HN, 128).T
        per_hg.append({
            "wk": np.ascontiguousarray(Wk[:, sl]),
            "wq": np.ascontiguousarray(Wq[:, sl]),
            "wv": np.ascontiguousarray(Wv[:, sl]),
            "wo": np.ascontiguousarray(Wo[sl, :]),
            "bkq": bkq,
            "bvb": np.ascontiguousarray(
                np.broadcast_to(bv[sl], (128, DHD))),
            "bob": (np.ascontiguousarray(np.broadcast_to(bo, (128, D)))
                    if hg == 0 else np.zeros((128, D), f32)),
            "mask": mask,
            "ones_c": ones_c,
        })

    in_maps = []
    for b in range(B):
        xt = np.ascontiguousarray(x[b].T)
        for hg in range(2):
            in_maps.append({"xt": xt, **per_hg[hg]})
    return in_maps


def run_cores(cfg, nc, in_maps, trace=False, tmpdir=None):
    c = _derived(cfg)
    n = c["N_CORES"]
    res = run_bass_kernel_spmd(
        nc, in_maps, list(range(n)), trace=trace, tmpdir=tmpdir
    )
    B, T, D = c["B"], c["T"], c["D"]
    out = np.empty((B, T, D), dtype=np.float32)
    for b in range(B):
        out[b] = res.results[2 * b]["o"]
        out[b] += res.results[2 * b + 1]["o"]
    return out, res


_NC_CACHE = {}


def kernel(x, Wq, bq, Wk, bk, Wv, bv, Wo, bo):
    cfg = PROD_CFG
    key = tuple(sorted(cfg.items()))
    if key not in _NC_CACHE:
        _NC_CACHE[key] = build_nc(cfg)
    nc = _NC_CACHE[key]
    inputs = dict(x=x, Wq=Wq, bq=bq, Wk=Wk, bk=bk, Wv=Wv, bv=bv, Wo=Wo, bo=bo)
    in_maps = make_core_inputs(cfg, inputs)
    out, _ = run_cores(cfg, nc, in_maps)
    return out


# revision 3
# speedup vs baseline: 1.7310x; 1.2105x over previous
"""Causal multi-head attention (B=4, T=2048, D=2048, H=16) on 8 Trainium2
NeuronCores via Bass/Tile, SPMD with zero collectives.

Sharding: core = (batch b, head-half hg). Each core owns one batch and 8 of
the 16 heads: it projects Q/K/V for its 1024-column slice of Wq/Wk/Wv over
the full sequence, runs causal attention for its 8 heads, and computes the
partial output projection A @ Wo[hg*1024:(hg+1)*1024, :]. The host feeds
x^T per batch (so no on-device transpose) and sums the two partials per
batch (bo is folded into the hg=0 partial on device via a broadcast tile).

All matmul operands are bf16 (inputs are cast on the host): bf16 runs the
PE at 1 cycle/row like f32r but its LDWEIGHTS uses the fast weight load
path (f32 cannot), halving the per-matmul weight-load overhead that
dominated the f32r version. Accumulation stays f32 in PSUM; softmax
denominators accumulate exactly via per-tile ones-vector matmuls into a
dedicated PSUM bank.

Per-core pipeline:
  1. K^T = Wk_s^T xT, Q^T = Wq_s^T xT (transposed layouts, bias fused into
     the PSUM evacuation), V = x Wv_s (natural layout, bias via broadcast
     tile) -> DRAM scratch. Weight slabs stay SBUF-resident; the t loop is
     outermost so compute starts as soon as the first x^T chunks land.
  2. per head: S^T supertiles (512 queries) = K^T_blk^T Q^T, exp on ACT
     (no max subtraction: scores are O(1)), causal diagonal masking by DVE
     multiply, A^T accumulated on PE with V as stationary, denominators
     on PE, normalization on the PSUM evacuation.
  3. O partial rows = A^T^T Wo_s (+ bo on hg=0 cores)
"""
import numpy as np
import ml_dtypes

import concourse.bacc as bacc
import concourse.mybir as mybir
from concourse.tile import TileContext
from concourse.bass_utils import run_bass_kernel_spmd

F32 = mybir.dt.float32
BF16 = mybir.dt.bfloat16
EXP = mybir.ActivationFunctionType.Exp
MULT = mybir.AluOpType.mult
ADD = mybir.AluOpType.add

PROD_CFG = dict(B=4, T=2048, D=2048, H=16)


def _derived(cfg):
    B, T, D, H = cfg["B"], cfg["T"], cfg["D"], cfg["H"]
    d = dict(cfg)
    d.update(
        HN=H // 2,            # heads per core
        DHD=(H // 2) * (D // H),  # local head dim total (1024)
        DK=D // 128,          # contraction chunks of x^T
        SS=512,               # query supertile width
        DH=D // H,            # 128
        N_CORES=2 * B,
    )
    return d


def build_nc(cfg):
    c = _derived(cfg)
    T, D = c["T"], c["D"]
    HN, DHD, DK, SS = c["HN"], c["DHD"], c["DK"], c["SS"]
    NB = T // 128          # key blocks (16)
    NST = T // SS          # supertiles (4)
    JPS = SS // 128        # key blocks per supertile (4)
    SCALE = float(c["DH"] ** -0.5)

    nc = bacc.Bacc(
        "TRN2", target_bir_lowering=False, debug=False, num_devices=c["N_CORES"]
    )
    xt = nc.dram_tensor("xt", [D, T], BF16, kind="ExternalInput").ap()
    wk = nc.dram_tensor("wk", [D, DHD], BF16, kind="ExternalInput").ap()
    wq = nc.dram_tensor("wq", [D, DHD], BF16, kind="ExternalInput").ap()
    wv = nc.dram_tensor("wv", [D, DHD], BF16, kind="ExternalInput").ap()
    wo = nc.dram_tensor("wo", [DHD, D], BF16, kind="ExternalInput").ap()
    bkq = nc.dram_tensor("bkq", [128, 2 * HN], F32, kind="ExternalInput").ap()
    bvb = nc.dram_tensor("bvb", [128, DHD], F32, kind="ExternalInput").ap()
    bob = nc.dram_tensor("bob", [128, D], F32, kind="ExternalInput").ap()
    mask = nc.dram_tensor("mask", [128, JPS * SS], BF16, kind="ExternalInput").ap()
    ones_c_in = nc.dram_tensor("ones_c", [128, 1], BF16, kind="ExternalInput").ap()
    o = nc.dram_tensor("o", [T, D], F32, kind="ExternalOutput").ap()

    kt_d = nc.dram_tensor("kt_scratch", [DHD, T], BF16).ap()
    qt_d = nc.dram_tensor("qt_scratch", [DHD, T], BF16).ap()
    v_d = nc.dram_tensor("v_scratch", [T, DHD], BF16).ap()

    with TileContext(nc) as tc:
        with tc.tile_pool(name="const", bufs=1) as pconst:
            ones_col = pconst.tile([128, 1], BF16, tag="ones_col")
            nc.sync.dma_start(out=ones_col[:], in_=ones_c_in[:])
            bkq_sb = pconst.tile([128, 2 * HN], F32, tag="bkq")
            nc.sync.dma_start(out=bkq_sb[:], in_=bkq[:])
            bv_sb = pconst.tile([128, DHD], F32, tag="bv")
            nc.sync.dma_start(out=bv_sb[:], in_=bvb[:])

            # ---------------- phase 1: K^T, Q^T, V ----------------
            with tc.tile_pool(name="slab", bufs=1) as pslab:
                slab = pslab.tile([128, DK * T], BF16, tag="slab")
                slab3 = slab[:].rearrange("p (k t) -> p k t", k=DK)
                xt3 = xt.rearrange("(k p) t -> p k t", p=128)
                # tq-major so the first projection tiles' inputs land first
                for tq in range(T // 512):
                    for k in range(DK):
                        nc.sync.dma_start(
                            out=slab3[:, k, tq * 512:(tq + 1) * 512],
                            in_=xt3[:, k, tq * 512:(tq + 1) * 512],
                        )

                # K^T and Q^T projections (transposed layout): the full
                # 8-head weight slab stays resident per projection
                with (
                    tc.tile_pool(name="p1w", bufs=2) as p1w,
                    tc.tile_pool(name="p1st", bufs=3) as p1st,
                    tc.tile_pool(name="ps_kq", bufs=2, space="PSUM") as ps_kq,
                ):
                    for pi, (w_in, out_d) in enumerate(((wk, kt_d), (wq, qt_d))):
                        w3 = w_in.rearrange("(k p) n -> p k n", p=128)
                        wall = p1w.tile([128, HN * DK * 128], BF16, tag="wall")
                        wall4 = wall[:].rearrange(
                            "p (m k n) -> p m k n", m=HN, k=DK
                        )
                        for m in range(HN):
                            nc.sync.dma_start(
                                out=wall4[:, m],
                                in_=w3[:, :, m * 128:(m + 1) * 128],
                            )
                        for tq in range(T // 512):
                            for m in range(HN):
                                ps = ps_kq.tile([128, 512], F32, tag="pskq")
                                for k in range(DK):
                                    nc.tensor.matmul(
                                        ps[:],
                                        wall4[:, m, k],
                                        slab3[:, k, tq * 512:(tq + 1) * 512],
                                        start=(k == 0),
                                        stop=(k == DK - 1),
                                    )
                                st = p1st.tile([128, 512], BF16, tag="kqst")
                                nc.vector.tensor_scalar_add(
                                    st[:], ps[:],
                                    bkq_sb[:, pi * HN + m:pi * HN + m + 1],
                                )
                                nc.sync.dma_start(
                                    out=out_d[m * 128:(m + 1) * 128,
                                              tq * 512:(tq + 1) * 512],
                                    in_=st[:],
                                )

                # V projection (natural layout), col halves of 512
                with (
                    tc.tile_pool(name="p1wv", bufs=2) as p1wv,
                    tc.tile_pool(name="p1vst", bufs=3) as p1vst,
                    tc.tile_pool(name="ps_v", bufs=2, space="PSUM") as ps_v,
                ):
                    wv3 = wv.rearrange("(k p) n -> p k n", p=128)
                    for cc in range(DHD // 512):
                        wvn = p1wv.tile([128, DK * 512], BF16, tag="wvn")
                        wvn3 = wvn[:].rearrange("p (k n) -> p k n", k=DK)
                        nc.sync.dma_start(
                            out=wvn[:], in_=wv3[:, :, cc * 512:(cc + 1) * 512]
                        )
                        for tb in range(NB):
                            ps = ps_v.tile([128, 512], F32, tag="psv")
                            for k in range(DK):
                                nc.tensor.matmul(
                                    ps[:],
                                    slab3[:, k, tb * 128:(tb + 1) * 128],
                                    wvn3[:, k],
                                    start=(k == 0),
                                    stop=(k == DK - 1),
                                )
                            st = p1vst.tile([128, 512], BF16, tag="vst")
                            nc.vector.tensor_tensor(
                                st[:], ps[:],
                                bv_sb[:, cc * 512:(cc + 1) * 512], ADD,
                            )
                            nc.sync.dma_start(
                                out=v_d[tb * 128:(tb + 1) * 128,
                                        cc * 512:(cc + 1) * 512],
                                in_=st[:],
                            )

            # ---------------- phase 2: attention per head ----------------
            with tc.tile_pool(name="aslab", bufs=1) as paslab:
                at_sb = paslab.tile([128, HN * T], BF16, tag="aslab")
                at3 = at_sb[:].rearrange("p (h t) -> p h t", h=HN)
                with (
                    tc.tile_pool(name="pmask", bufs=1) as pmask,
                    tc.tile_pool(name="ph", bufs=2) as ph,
                    tc.tile_pool(name="pe", bufs=4) as pe_pool,
                    tc.tile_pool(name="paraw", bufs=4) as paraw,
                    tc.tile_pool(name="psm", bufs=4) as psm,
                    tc.tile_pool(name="plb", bufs=4) as plb,
                    tc.tile_pool(name="ps_s", bufs=3, space="PSUM") as ps_s,
                    tc.tile_pool(name="ps_a", bufs=2, space="PSUM") as ps_a,
                    tc.tile_pool(name="ps_l", bufs=2, space="PSUM") as ps_l,
                ):
                    mask_sb = pmask.tile([128, JPS * SS], BF16, tag="mask")
                    nc.sync.dma_start(out=mask_sb[:], in_=mask[:])
                    v_dr = v_d.rearrange("(jb p) c -> p jb c", p=128)
                    for h in range(HN):
                        kt_h = ph.tile([128, T], BF16, tag="kth")
                        nc.sync.dma_start(
                            out=kt_h[:], in_=kt_d[h * 128:(h + 1) * 128, :]
                        )
                        qt_h = ph.tile([128, T], BF16, tag="qth")
                        nc.sync.dma_start(
                            out=qt_h[:], in_=qt_d[h * 128:(h + 1) * 128, :]
                        )
                        v_h = ph.tile([128, NB * 128], BF16, tag="vh")
                        v_h3 = v_h[:].rearrange("p (j c) -> p j c", j=NB)
                        nc.sync.dma_start(
                            out=v_h3,
                            in_=v_dr[:, :, h * 128:(h + 1) * 128],
                        )

                        def finish_supertile(s, psa, psl):
                            # copy accumulators out fast (frees the PSUM
                            # banks), then the reciprocal/normalize chain
                            araw = paraw.tile([128, SS], F32, tag="araw")
                            nc.vector.tensor_copy(araw[:], psa[:])
                            l_sb = psm.tile([1, SS], F32, tag="lsb")
                            nc.vector.tensor_copy(l_sb[:], psl[:])
                            linv = psm.tile([1, SS], F32, tag="linv")
                            nc.vector.reciprocal_approx_fast(linv[:], l_sb[:])
                            lb = plb.tile([128, SS], F32, tag="lb")
                            nc.gpsimd.partition_broadcast(
                                lb[:], linv[:], channels=128
                            )
                            nc.vector.tensor_tensor(
                                at3[:, h, s * SS:(s + 1) * SS],
                                araw[:], lb[:], MULT,
                            )

                        pending = None
                        for s in range(NST):
                            psa = ps_a.tile([128, SS], F32, tag="psa")
                            psl = ps_l.tile([1, SS], F32, tag="psl")
                            nj = JPS * s + JPS
                            for j in range(nj):
                                pss = ps_s.tile([128, SS], F32, tag="pss")
                                nc.tensor.matmul(
                                    pss[:],
                                    kt_h[:, j * 128:(j + 1) * 128],
                                    qt_h[:, s * SS:(s + 1) * SS],
                                    start=True, stop=True,
                                )
                                et = pe_pool.tile([128, SS], BF16, tag="et")
                                nc.scalar.activation(
                                    et[:], pss[:], EXP, scale=SCALE
                                )
                                if j >= JPS * s:
                                    mc = (j - JPS * s) * SS
                                    nc.vector.tensor_mul(
                                        et[:], et[:], mask_sb[:, mc:mc + SS]
                                    )
                                if pending is not None:
                                    pending()
                                vj = v_h3[:, j, :]
                                first, last = (j == 0), (j == nj - 1)

                                def consume(et=et, vj=vj, first=first,
                                            last=last, psa=psa, psl=psl,
                                            s=s):
                                    nc.tensor.matmul(
                                        psa[:], vj, et[:],
                                        start=first, stop=last,
                                    )
                                    nc.tensor.matmul(
                                        psl[:], ones_col[:], et[:],
                                        start=first, stop=last,
                                    )
                                    if last:
                                        finish_supertile(s, psa, psl)

                                pending = consume
                        pending()
                        pending = None

                # ---------------- phase 3: output projection ----------------
                with (
                    tc.tile_pool(name="p3w", bufs=1) as p3w,
                    tc.tile_pool(name="p3b", bufs=1) as p3b,
                    tc.tile_pool(name="p3st", bufs=3) as p3st,
                    tc.tile_pool(name="ps_o", bufs=2, space="PSUM") as ps_o,
                ):
                    bo_sb = p3b.tile([128, D], F32, tag="bo")
                    nc.sync.dma_start(out=bo_sb[:], in_=bob[:])
                    wo_sb = p3w.tile([128, HN * D], BF16, tag="wo")
                    wo3 = wo_sb[:].rearrange("p (h n) -> p h n", h=HN)
                    wo_r = wo.rearrange("(k p) n -> p k n", p=128)
                    for h in range(HN):
                        nc.sync.dma_start(out=wo3[:, h], in_=wo_r[:, h])
                    for tb in range(NB):
                        for cc in range(D // 512):
                            pso = ps_o.tile([128, 512], F32, tag="pso")
                            for h in range(HN):
                                nc.tensor.matmul(
                                    pso[:],
                                    at3[:, h, tb * 128:(tb + 1) * 128],
                                    wo3[:, h, cc * 512:(cc + 1) * 512],
                                    start=(h == 0),
                                    stop=(h == HN - 1),
                                )
                            ost = p3st.tile([128, 512], F32, tag="ost")
                            nc.vector.tensor_tensor(
                                ost[:], pso[:],
                                bo_sb[:, cc * 512:(cc + 1) * 512], ADD,
                            )
                            nc.sync.dma_start(
                                out=o[tb * 128:(tb + 1) * 128,
                                      cc * 512:(cc + 1) * 512],
                                in_=ost[:],
                            )
    nc.compile()
    return nc


def make_core_inputs(cfg, inputs):
    """Per-core input maps. Core index = 2*b + hg."""
    c = _derived(cfg)
    B, T, D, H = c["B"], c["T"], c["D"], c["H"]
    HN, DHD, SS = c["HN"], c["DHD"], c["SS"]
    JPS = SS // 128
    f32 = np.float32
    bf16 = ml_dtypes.bfloat16
    x = np.asarray(inputs["x"], f32)
    Wk = np.asarray(inputs["Wk"], f32)
    Wq = np.asarray(inputs["Wq"], f32)
    Wv = np.asarray(inputs["Wv"], f32)
    Wo = np.asarray(inputs["Wo"], f32)
    bk = np.asarray(inputs["bk"], f32)
    bq = np.asarray(inputs["bq"], f32)
    bv = np.asarray(inputs["bv"], f32)
    bo = np.asarray(inputs["bo"], f32)

    p = np.arange(128)[:, None]
    cq = np.arange(SS)[None, :]
    mask = np.empty((128, JPS * SS), dtype=bf16)
    for jj in range(JPS):
        mask[:, jj * SS:(jj + 1) * SS] = (jj * 128 + p <= cq).astype(bf16)
    ones_c = np.ones((128, 1), bf16)

    per_hg = []
    for hg in range(2):
        sl = slice(hg * DHD, (hg + 1) * DHD)
        bkq = np.empty((128, 2 * HN), f32)
        bkq[:, :HN] = bk[sl].reshape(HN, 128).T
        bkq[:, HN:] = bq[sl].reshape(HN, 128).T
        per_hg.append({
            "wk": np.ascontiguousarray(Wk[:, sl]).astype(bf16),
            "wq": np.ascontiguousarray(Wq[:, sl]).astype(bf16),
            "wv": np.ascontiguousarray(Wv[:, sl]).astype(bf16),
            "wo": np.ascontiguousarray(Wo[sl, :]).astype(bf16),
            "bkq": bkq,
            "bvb": np.ascontiguousarray(
                np.broadcast_to(bv[sl], (128, DHD))),
            "bob": (np.ascontiguousarray(np.broadcast_to(bo, (128, D)))
                    if hg == 0 else np.zeros((128, D), f32)),
            "mask": mask,
            "ones_c": ones_c,
        })

    in_maps = []
    for b in range(B):
        xt = np.ascontiguousarray(x[b].T).astype(bf16)
        for hg in range(2):
            in_maps.append({"xt": xt, **per_hg[hg]})
    return in_maps


def run_cores(cfg, nc, in_maps, trace=False, tmpdir=None):
    c = _derived(cfg)
    n = c["N_CORES"]
    res = run_bass_kernel_spmd(
        nc, in_maps, list(range(n)), trace=trace, tmpdir=tmpdir
    )
    B, T, D = c["B"], c["T"], c["D"]
    out = np.empty((B, T, D), dtype=np.float32)
    for b in range(B):
        out[b] = res.results[2 * b]["o"]
        out[b] += res.results[2 * b + 1]["o"]
    return out, res


_NC_CACHE = {}


def kernel(x, Wq, bq, Wk, bk, Wv, bv, Wo, bo):
    cfg = PROD_CFG
    key = tuple(sorted(cfg.items()))
    if key not in _NC_CACHE:
        _NC_CACHE[key] = build_nc(cfg)
    nc = _NC_CACHE[key]
    inputs = dict(x=x, Wq=Wq, bq=bq, Wk=Wk, bk=bk, Wv=Wv, bv=bv, Wo=Wo, bo=bo)
    in_maps = make_core_inputs(cfg, inputs)
    out, _ = run_cores(cfg, nc, in_maps)
    return out


# revision 4
# speedup vs baseline: 1.8773x; 1.0845x over previous
"""Causal multi-head attention (B=4, T=2048, D=2048, H=16) on 8 Trainium2
NeuronCores via Bass/Tile, SPMD with zero collectives.

Sharding: core = (batch b, head-half hg). Each core owns one batch and 8 of
the 16 heads: it projects Q/K/V for its 1024-column slice of Wq/Wk/Wv over
the full sequence, runs causal attention for its 8 heads, and computes the
partial output projection A @ Wo[hg*1024:(hg+1)*1024, :]. The host feeds
x^T per batch (so no on-device transpose) and sums the two partials per
batch (bo is folded into the hg=0 partial on device via a broadcast tile).

All matmul operands are bf16 (cast on the host): bf16 runs the PE at 1
cycle/row like f32r but its LDWEIGHTS uses the fast weight load path (f32
cannot), halving per-matmul weight-load overhead. Accumulation stays f32
in PSUM; softmax denominators accumulate exactly via per-tile ones-vector
matmuls into a dedicated PSUM bank.

DMA is split across the two hardware queues: x^T slab and scratch spills
ride the SP queue while weight slabs / per-head K,Q,V reloads ride the
Activation queue, so the projection weights land in parallel with x^T and
compute starts within a few us.

Per-core pipeline:
  1. K^T = Wk_s^T xT, Q^T = Wq_s^T xT (transposed layouts, bias fused into
     the PSUM evacuation), V = x Wv_s (natural layout, bias via broadcast
     tile) -> DRAM scratch. Weight slabs stay SBUF-resident; the t loop is
     outermost so compute starts as soon as the first x^T chunks land.
  2. per head: S^T supertiles (512 queries) = K^T_blk^T Q^T; diagonal key
     blocks use restricted query ranges (moving dim 512-128*jj) so only a
     single fixed 128x128 triangle mask is ever applied; exp on ACT (no
     max subtraction: scores are O(1)); A^T and the softmax denominators
     accumulate on PE; normalization on the PSUM evacuation.
  3. O partial rows = A^T^T Wo_s (+ bo on hg=0 cores), Wo prefetched
     during phase 2.
"""
import numpy as np
import ml_dtypes

import concourse.bacc as bacc
import concourse.mybir as mybir
from concourse.tile import TileContext
from concourse.bass_utils import run_bass_kernel_spmd

F32 = mybir.dt.float32
BF16 = mybir.dt.bfloat16
EXP = mybir.ActivationFunctionType.Exp
MULT = mybir.AluOpType.mult
ADD = mybir.AluOpType.add

PROD_CFG = dict(B=4, T=2048, D=2048, H=16)


def _derived(cfg):
    B, T, D, H = cfg["B"], cfg["T"], cfg["D"], cfg["H"]
    d = dict(cfg)
    d.update(
        HN=H // 2,            # heads per core
        DHD=(H // 2) * (D // H),  # local head dim total (1024)
        DK=D // 128,          # contraction chunks of x^T
        SS=512,               # query supertile width
        DH=D // H,            # 128
        N_CORES=2 * B,
    )
    return d


def build_nc(cfg):
    c = _derived(cfg)
    T, D = c["T"], c["D"]
    HN, DHD, DK, SS = c["HN"], c["DHD"], c["DK"], c["SS"]
    NB = T // 128          # key blocks (16)
    NST = T // SS          # supertiles (4)
    JPS = SS // 128        # key blocks per supertile (4)
    SCALE = float(c["DH"] ** -0.5)

    nc = bacc.Bacc(
        "TRN2", target_bir_lowering=False, debug=False, num_devices=c["N_CORES"]
    )
    xt = nc.dram_tensor("xt", [D, T], BF16, kind="ExternalInput").ap()
    wk = nc.dram_tensor("wk", [D, DHD], BF16, kind="ExternalInput").ap()
    wq = nc.dram_tensor("wq", [D, DHD], BF16, kind="ExternalInput").ap()
    wv = nc.dram_tensor("wv", [D, DHD], BF16, kind="ExternalInput").ap()
    wo = nc.dram_tensor("wo", [DHD, D], BF16, kind="ExternalInput").ap()
    bkq = nc.dram_tensor("bkq", [128, 2 * HN], F32, kind="ExternalInput").ap()
    bvb = nc.dram_tensor("bvb", [128, DHD], F32, kind="ExternalInput").ap()
    bob = nc.dram_tensor("bob", [128, D], F32, kind="ExternalInput").ap()
    mask = nc.dram_tensor("mask", [128, 128], BF16, kind="ExternalInput").ap()
    ones_c_in = nc.dram_tensor("ones_c", [128, 1], BF16, kind="ExternalInput").ap()
    o = nc.dram_tensor("o", [T, D], F32, kind="ExternalOutput").ap()

    kt_d = nc.dram_tensor("kt_scratch", [DHD, T], BF16).ap()
    qt_d = nc.dram_tensor("qt_scratch", [DHD, T], BF16).ap()
    v_d = nc.dram_tensor("v_scratch", [T, DHD], BF16).ap()

    with TileContext(nc) as tc:
        with tc.tile_pool(name="const", bufs=1) as pconst:
            ones_col = pconst.tile([128, 1], BF16, tag="ones_col")
            nc.scalar.dma_start(out=ones_col[:], in_=ones_c_in[:])
            bkq_sb = pconst.tile([128, 2 * HN], F32, tag="bkq")
            nc.scalar.dma_start(out=bkq_sb[:], in_=bkq[:])
            bv_sb = pconst.tile([128, DHD], F32, tag="bv")
            nc.scalar.dma_start(out=bv_sb[:], in_=bvb[:])

            # ---------------- phase 1: K^T, Q^T, V ----------------
            with (
                tc.tile_pool(name="slab", bufs=1) as pslab,
                tc.tile_pool(name="p1w", bufs=2) as p1w,
                tc.tile_pool(name="p1wv", bufs=2) as p1wv,
            ):
                slab = pslab.tile([128, DK * T], BF16, tag="slab")
                slab3 = slab[:].rearrange("p (k t) -> p k t", k=DK)
                xt3 = xt.rearrange("(k p) t -> p k t", p=128)

                # weight slabs ride the ACT queue, x^T rides the SP queue:
                # both land in parallel and compute starts within a few us
                walls = []
                for w_in in (wk, wq):
                    w3 = w_in.rearrange("(k p) n -> p k n", p=128)
                    wall = p1w.tile([128, HN * DK * 128], BF16, tag="wall")
                    wall4 = wall[:].rearrange(
                        "p (m k n) -> p m k n", m=HN, k=DK
                    )
                    walls.append(wall4)
                    for m in range(HN):
                        nc.scalar.dma_start(
                            out=wall4[:, m],
                            in_=w3[:, :, m * 128:(m + 1) * 128],
                        )
                # tq-major so the first projection tiles' inputs land first
                for tq in range(T // 512):
                    for k in range(DK):
                        nc.sync.dma_start(
                            out=slab3[:, k, tq * 512:(tq + 1) * 512],
                            in_=xt3[:, k, tq * 512:(tq + 1) * 512],
                        )
                # V weights prefetch (consumed after K/Q projections)
                wv3 = wv.rearrange("(k p) n -> p k n", p=128)
                wvns = []
                for cc in range(DHD // 512):
                    wvn = p1wv.tile([128, DK * 512], BF16, tag="wvn")
                    wvns.append(wvn[:].rearrange("p (k n) -> p k n", k=DK))
                    nc.scalar.dma_start(
                        out=wvn[:], in_=wv3[:, :, cc * 512:(cc + 1) * 512]
                    )

                with (
                    tc.tile_pool(name="p1st", bufs=3) as p1st,
                    tc.tile_pool(name="ps_kq", bufs=2, space="PSUM") as ps_kq,
                ):
                    for pi, out_d in enumerate((kt_d, qt_d)):
                        wall4 = walls[pi]
                        for tq in range(T // 512):
                            for m in range(HN):
                                ps = ps_kq.tile([128, 512], F32, tag="pskq")
                                for k in range(DK):
                                    nc.tensor.matmul(
                                        ps[:],
                                        wall4[:, m, k],
                                        slab3[:, k, tq * 512:(tq + 1) * 512],
                                        start=(k == 0),
                                        stop=(k == DK - 1),
                                    )
                                st = p1st.tile([128, 512], BF16, tag="kqst")
                                nc.vector.tensor_scalar_add(
                                    st[:], ps[:],
                                    bkq_sb[:, pi * HN + m:pi * HN + m + 1],
                                )
                                nc.sync.dma_start(
                                    out=out_d[m * 128:(m + 1) * 128,
                                              tq * 512:(tq + 1) * 512],
                                    in_=st[:],
                                )

                # V projection (natural layout), col halves of 512
                with (
                    tc.tile_pool(name="p1vst", bufs=3) as p1vst,
                    tc.tile_pool(name="ps_v", bufs=2, space="PSUM") as ps_v,
                ):
                    for cc in range(DHD // 512):
                        wvn3 = wvns[cc]
                        for tb in range(NB):
                            ps = ps_v.tile([128, 512], F32, tag="psv")
                            for k in range(DK):
                                nc.tensor.matmul(
                                    ps[:],
                                    slab3[:, k, tb * 128:(tb + 1) * 128],
                                    wvn3[:, k],
                                    start=(k == 0),
                                    stop=(k == DK - 1),
                                )
                            st = p1vst.tile([128, 512], BF16, tag="vst")
                            nc.vector.tensor_tensor(
                                st[:], ps[:],
                                bv_sb[:, cc * 512:(cc + 1) * 512], ADD,
                            )
                            nc.sync.dma_start(
                                out=v_d[tb * 128:(tb + 1) * 128,
                                        cc * 512:(cc + 1) * 512],
                                in_=st[:],
                            )

            # ---------------- phase 2: attention per head ----------------
            with (
                tc.tile_pool(name="aslab", bufs=1) as paslab,
                tc.tile_pool(name="p3w", bufs=1) as p3w,
                tc.tile_pool(name="p3b", bufs=1) as p3b,
            ):
                at_sb = paslab.tile([128, HN * T], BF16, tag="aslab")
                at3 = at_sb[:].rearrange("p (h t) -> p h t", h=HN)
                with (
                    tc.tile_pool(name="pmask", bufs=1) as pmask,
                    tc.tile_pool(name="ph", bufs=2) as ph,
                    tc.tile_pool(name="pe", bufs=4) as pe_pool,
                    tc.tile_pool(name="paraw", bufs=4) as paraw,
                    tc.tile_pool(name="psm", bufs=4) as psm,
                    tc.tile_pool(name="plb", bufs=4) as plb,
                    tc.tile_pool(name="ps_s", bufs=3, space="PSUM") as ps_s,
                    tc.tile_pool(name="ps_a", bufs=2, space="PSUM") as ps_a,
                    tc.tile_pool(name="ps_l", bufs=2, space="PSUM") as ps_l,
                ):
                    mask_sb = pmask.tile([128, 128], BF16, tag="mask")
                    nc.scalar.dma_start(out=mask_sb[:], in_=mask[:])
                    v_dr = v_d.rearrange("(jb p) c -> p jb c", p=128)
                    for h in range(HN):
                        # head 0/1 loads ride the ACT queue (idle at the
                        # phase boundary; SP still drains V spills)
                        eng = nc.scalar if h < 2 else nc.sync
                        kt_h = ph.tile([128, T], BF16, tag="kth")
                        eng.dma_start(
                            out=kt_h[:], in_=kt_d[h * 128:(h + 1) * 128, :]
                        )
                        qt_h = ph.tile([128, T], BF16, tag="qth")
                        eng.dma_start(
                            out=qt_h[:], in_=qt_d[h * 128:(h + 1) * 128, :]
                        )
                        v_h = ph.tile([128, NB * 128], BF16, tag="vh")
                        v_h3 = v_h[:].rearrange("p (j c) -> p j c", j=NB)
                        eng.dma_start(
                            out=v_h3,
                            in_=v_dr[:, :, h * 128:(h + 1) * 128],
                        )
                        if h == 0:
                            # phase-3 weight prefetch, behind head-0 loads
                            bo_sb = p3b.tile([128, D], F32, tag="bo")
                            nc.scalar.dma_start(out=bo_sb[:], in_=bob[:])
                            wo_sb = p3w.tile([128, HN * D], BF16, tag="wo")
                            wo3 = wo_sb[:].rearrange("p (h n) -> p h n", h=HN)
                            wo_r = wo.rearrange("(k p) n -> p k n", p=128)
                            for hh in range(HN):
                                nc.scalar.dma_start(
                                    out=wo3[:, hh], in_=wo_r[:, hh]
                                )

                        def finish_supertile(s, psa, psl):
                            # copy accumulators out fast (frees the PSUM
                            # banks), then the reciprocal/normalize chain
                            araw = paraw.tile([128, SS], F32, tag="araw")
                            nc.vector.tensor_copy(araw[:], psa[:])
                            l_sb = psm.tile([1, SS], F32, tag="lsb")
                            nc.vector.tensor_copy(l_sb[:], psl[:])
                            linv = psm.tile([1, SS], F32, tag="linv")
                            nc.vector.reciprocal_approx_fast(linv[:], l_sb[:])
                            lb = plb.tile([128, SS], F32, tag="lb")
                            nc.gpsimd.partition_broadcast(
                                lb[:], linv[:], channels=128
                            )
                            nc.vector.tensor_tensor(
                                at3[:, h, s * SS:(s + 1) * SS],
                                araw[:], lb[:], MULT,
                            )

                        pending = None
                        for s in range(NST):
                            psa = ps_a.tile([128, SS], F32, tag="psa")
                            psl = ps_l.tile([1, SS], F32, tag="psl")
                            nj = JPS * s + JPS
                            for j in range(nj):
                                # diagonal key blocks only score the
                                # queries they can see
                                off = max(0, (j - JPS * s) * 128)
                                w = SS - off
                                pss = ps_s.tile([128, SS], F32, tag="pss")
                                nc.tensor.matmul(
                                    pss[:, :w],
                                    kt_h[:, j * 128:(j + 1) * 128],
                                    qt_h[:, s * SS + off:(s + 1) * SS],
                                    start=True, stop=True,
                                )
                                et = pe_pool.tile([128, SS], BF16, tag="et")
                                nc.scalar.activation(
                                    et[:, :w], pss[:, :w], EXP, scale=SCALE
                                )
                                if j >= JPS * s:
                                    nc.vector.tensor_mul(
                                        et[:, :128], et[:, :128], mask_sb[:]
                                    )
                                if pending is not None:
                                    pending()
                                vj = v_h3[:, j, :]
                                first, last = (j == 0), (j == nj - 1)

                                def consume(et=et, vj=vj, first=first,
                                            last=last, psa=psa, psl=psl,
                                            s=s, off=off, w=w):
                                    nc.tensor.matmul(
                                        psa[:, off:off + w], vj, et[:, :w],
                                        start=first, stop=last,
                                    )
                                    nc.tensor.matmul(
                                        psl[:, off:off + w], ones_col[:],
                                        et[:, :w],
                                        start=first, stop=last,
                                    )
                                    if last:
                                        finish_supertile(s, psa, psl)

                                pending = consume
                        pending()
                        pending = None

                # ---------------- phase 3: output projection ----------------
                with (
                    tc.tile_pool(name="p3st", bufs=3) as p3st,
                    tc.tile_pool(name="ps_o", bufs=2, space="PSUM") as ps_o,
                ):
                    for tb in range(NB):
                        for cc in range(D // 512):
                            pso = ps_o.tile([128, 512], F32, tag="pso")
                            for hh in range(HN):
                                nc.tensor.matmul(
                                    pso[:],
                                    at3[:, hh, tb * 128:(tb + 1) * 128],
                                    wo3[:, hh, cc * 512:(cc + 1) * 512],
                                    start=(hh == 0),
                                    stop=(hh == HN - 1),
                                )
                            ost = p3st.tile([128, 512], F32, tag="ost")
                            nc.vector.tensor_tensor(
                                ost[:], pso[:],
                                bo_sb[:, cc * 512:(cc + 1) * 512], ADD,
                            )
                            nc.sync.dma_start(
                                out=o[tb * 128:(tb + 1) * 128,
                                      cc * 512:(cc + 1) * 512],
                                in_=ost[:],
                            )
    nc.compile()
    return nc


def make_core_inputs(cfg, inputs):
    """Per-core input maps. Core index = 2*b + hg."""
    c = _derived(cfg)
    B, T, D, H = c["B"], c["T"], c["D"], c["H"]
    HN, DHD = c["HN"], c["DHD"]
    f32 = np.float32
    bf16 = ml_dtypes.bfloat16
    x = np.asarray(inputs["x"], f32)
    Wk = np.asarray(inputs["Wk"], f32)
    Wq = np.asarray(inputs["Wq"], f32)
    Wv = np.asarray(inputs["Wv"], f32)
    Wo = np.asarray(inputs["Wo"], f32)
    bk = np.asarray(inputs["bk"], f32)
    bq = np.asarray(inputs["bq"], f32)
    bv = np.asarray(inputs["bv"], f32)
    bo = np.asarray(inputs["bo"], f32)

    p = np.arange(128)[:, None]
    cq = np.arange(128)[None, :]
    mask = (p <= cq).astype(bf16)
    ones_c = np.ones((128, 1), bf16)

    per_hg = []
    for hg in range(2):
        sl = slice(hg * DHD, (hg + 1) * DHD)
        bkq = np.empty((128, 2 * HN), f32)
        bkq[:, :HN] = bk[sl].reshape(HN, 128).T
        bkq[:, HN:] = bq[sl].reshape(HN, 128).T
        per_hg.append({
            "wk": np.ascontiguousarray(Wk[:, sl]).astype(bf16),
            "wq": np.ascontiguousarray(Wq[:, sl]).astype(bf16),
            "wv": np.ascontiguousarray(Wv[:, sl]).astype(bf16),
            "wo": np.ascontiguousarray(Wo[sl, :]).astype(bf16),
            "bkq": bkq,
            "bvb": np.ascontiguousarray(
                np.broadcast_to(bv[sl], (128, DHD))),
            "bob": (np.ascontiguousarray(np.broadcast_to(bo, (128, D)))
                    if hg == 0 else np.zeros((128, D), f32)),
            "mask": mask,
            "ones_c": ones_c,
        })

    in_maps = []
    for b in range(B):
        xt = np.ascontiguousarray(x[b].T).astype(bf16)
        for hg in range(2):
            in_maps.append({"xt": xt, **per_hg[hg]})
    return in_maps


def run_cores(cfg, nc, in_maps, trace=False, tmpdir=None):
    c = _derived(cfg)
    n = c["N_CORES"]
    res = run_bass_kernel_spmd(
        nc, in_maps, list(range(n)), trace=trace, tmpdir=tmpdir
    )
    B, T, D = c["B"], c["T"], c["D"]
    out = np.empty((B, T, D), dtype=np.float32)
    for b in range(B):
        out[b] = res.results[2 * b]["o"]
        out[b] += res.results[2 * b + 1]["o"]
    return out, res


_NC_CACHE = {}


def kernel(x, Wq, bq, Wk, bk, Wv, bv, Wo, bo):
    cfg = PROD_CFG
    key = tuple(sorted(cfg.items()))
    if key not in _NC_CACHE:
        _NC_CACHE[key] = build_nc(cfg)
    nc = _NC_CACHE[key]
    inputs = dict(x=x, Wq=Wq, bq=bq, Wk=Wk, bk=bk, Wv=Wv, bv=bv, Wo=Wo, bo=bo)
    in_maps = make_core_inputs(cfg, inputs)
    out, _ = run_cores(cfg, nc, in_maps)
    return out


# revision 7
# speedup vs baseline: 1.9106x; 1.0178x over previous
"""Causal multi-head attention (B=4, T=2048, D=2048, H=16) on 8 Trainium2
NeuronCores via Bass/Tile, SPMD with zero collectives.

Sharding: core = (batch b, head-half hg). Each core owns one batch and 8 of
the 16 heads: it projects Q/K/V for its 1024-column slice of Wq/Wk/Wv over
the full sequence, runs causal attention for its 8 heads, and computes the
partial output projection A @ Wo[hg*1024:(hg+1)*1024, :]. The host feeds
x^T per batch and sums the two partials per batch (bo is folded into the
hg=0 partial on device via a broadcast tile).

All matmul operands are bf16 (cast on the host): bf16 runs the PE at 1
cycle/row like f32r but its LDWEIGHTS uses the fast weight load path (f32
cannot), halving per-matmul weight-load overhead. Accumulation stays f32
in PSUM; softmax denominators accumulate exactly via per-tile ones-vector
matmuls into a dedicated PSUM bank.

The host pre-arranges x^T and every weight into the exact SBUF slab layout
([partition, chunk, col]), so each slab arrives as one full-bandwidth
contiguous DMA. DMA is split across the two hardware queues (SP and
Activation) so weights land in parallel with x^T and compute starts within
a few us.

Per-core pipeline:
  1. K^T = Wk_s^T xT, Q^T = Wq_s^T xT (transposed layouts, bias fused into
     the PSUM evacuation), V = x Wv_s (natural layout, bias via broadcast
     tile) -> DRAM scratch.
  2. per head: S^T supertiles (512 queries) = K^T_blk^T Q^T; diagonal key
     blocks use restricted query ranges (moving dim 512-128*jj) so only a
     single fixed 128x128 triangle mask is ever applied; exp on ACT (no
     max subtraction: scores are O(1)); A^T and the softmax denominators
     accumulate on PE; normalization on the PSUM evacuation. The
     S -> exp -> (mask) -> AV chain is software-pipelined two units deep
     to absorb cross-engine semaphore latency.
  3. O partial rows = A^T^T Wo_s (+ bo on hg=0 cores), Wo prefetched
     during phase 2.
"""
import numpy as np
import ml_dtypes

import concourse.bacc as bacc
import concourse.mybir as mybir
from concourse.tile import TileContext
from concourse.bass_utils import run_bass_kernel_spmd

F32 = mybir.dt.float32
BF16 = mybir.dt.bfloat16
EXP = mybir.ActivationFunctionType.Exp
MULT = mybir.AluOpType.mult
ADD = mybir.AluOpType.add

PROD_CFG = dict(B=4, T=2048, D=2048, H=16)


def _derived(cfg):
    B, T, D, H = cfg["B"], cfg["T"], cfg["D"], cfg["H"]
    d = dict(cfg)
    d.update(
        HN=H // 2,            # heads per core
        DHD=(H // 2) * (D // H),  # local head dim total (1024)
        DK=D // 128,          # contraction chunks of x^T
        SS=512,               # query supertile width
        DH=D // H,            # 128
        N_CORES=2 * B,
    )
    return d


def build_nc(cfg):
    c = _derived(cfg)
    T, D = c["T"], c["D"]
    HN, DHD, DK, SS = c["HN"], c["DHD"], c["DK"], c["SS"]
    NB = T // 128          # key blocks (16)
    NST = T // SS          # supertiles (4)
    JPS = SS // 128        # key blocks per supertile (4)
    SCALE = float(c["DH"] ** -0.5)

    nc = bacc.Bacc(
        "TRN2", target_bir_lowering=False, debug=False, num_devices=c["N_CORES"]
    )
    # host-prearranged slab layouts: [128, chunk*cols] contiguous
    xt = nc.dram_tensor("xt", [128, DK * T], BF16, kind="ExternalInput").ap()
    wkq = nc.dram_tensor(
        "wkq", [128, 2 * HN * DK * 128], BF16, kind="ExternalInput"
    ).ap()
    wv = nc.dram_tensor("wv", [128, DK * DHD], BF16, kind="ExternalInput").ap()
    wo = nc.dram_tensor("wo", [128, HN * D], BF16, kind="ExternalInput").ap()
    bkq = nc.dram_tensor("bkq", [128, 2 * HN], F32, kind="ExternalInput").ap()
    bvb = nc.dram_tensor("bvb", [128, DHD], F32, kind="ExternalInput").ap()
    bob = nc.dram_tensor("bob", [128, D], F32, kind="ExternalInput").ap()
    mask = nc.dram_tensor("mask", [128, 128], BF16, kind="ExternalInput").ap()
    ones_c_in = nc.dram_tensor("ones_c", [128, 1], BF16, kind="ExternalInput").ap()
    o = nc.dram_tensor("o", [T, D], F32, kind="ExternalOutput").ap()

    kt_d = nc.dram_tensor("kt_scratch", [DHD, T], BF16).ap()
    qt_d = nc.dram_tensor("qt_scratch", [DHD, T], BF16).ap()
    # per-half V scratch so head-0's reload isn't gated on the full tensor
    v_ds = [
        nc.dram_tensor(f"v_scratch{i}", [T, DHD // 2], BF16).ap()
        for i in range(2)
    ]

    with TileContext(nc) as tc:
        with tc.tile_pool(name="const", bufs=1) as pconst:
            ones_col = pconst.tile([128, 1], BF16, tag="ones_col")
            nc.scalar.dma_start(out=ones_col[:], in_=ones_c_in[:])
            bkq_sb = pconst.tile([128, 2 * HN], F32, tag="bkq")
            nc.scalar.dma_start(out=bkq_sb[:], in_=bkq[:])
            bv_sb = pconst.tile([128, DHD], F32, tag="bv")
            nc.scalar.dma_start(out=bv_sb[:], in_=bvb[:])

            # ---------------- phase 1: K^T, Q^T, V ----------------
            with (
                tc.tile_pool(name="slab", bufs=1) as pslab,
                tc.tile_pool(name="p1w", bufs=2) as p1w,
                tc.tile_pool(name="p1wv", bufs=2) as p1wv,
            ):
                slab = pslab.tile([128, DK * T], BF16, tag="slab")
                # slab stored tq-major: [p, tq, k, 512]
                slab4 = slab[:].rearrange(
                    "p (tq k t) -> p tq k t", tq=T // 512, k=DK
                )
                xt4 = xt.rearrange("p (tq k t) -> p tq k t", tq=T // 512, k=DK)

                # weight slabs ride the ACT queue, x^T rides the SP queue
                walls = []
                wkq2 = wkq.rearrange("p (i r) -> p i r", i=2)
                for i in range(2):
                    wall = p1w.tile([128, HN * DK * 128], BF16, tag="wall")
                    walls.append(
                        wall[:].rearrange("p (m k n) -> p m k n", m=HN, k=DK)
                    )
                    nc.scalar.dma_start(out=wall[:], in_=wkq2[:, i])
                for tq in range(T // 512):
                    nc.sync.dma_start(out=slab4[:, tq], in_=xt4[:, tq])
                # V weights prefetch (consumed after K/Q projections)
                wv3 = wv.rearrange("p (cc r) -> p cc r", cc=2)
                wvns = []
                for cc in range(DHD // 512):
                    wvn = p1wv.tile([128, DK * 512], BF16, tag="wvn")
                    wvns.append(wvn[:].rearrange("p (k n) -> p k n", k=DK))
                    nc.scalar.dma_start(out=wvn[:], in_=wv3[:, cc])

                with (
                    tc.tile_pool(name="p1st", bufs=3) as p1st,
                    tc.tile_pool(name="ps_kq", bufs=2, space="PSUM") as ps_kq,
                ):
                    for pi, out_d in enumerate((kt_d, qt_d)):
                        wall4 = walls[pi]
                        for tq in range(T // 512):
                            for m in range(HN):
                                ps = ps_kq.tile([128, 512], F32, tag="pskq")
                                for k in range(DK):
                                    nc.tensor.matmul(
                                        ps[:],
                                        wall4[:, m, k],
                                        slab4[:, tq, k],
                                        start=(k == 0),
                                        stop=(k == DK - 1),
                                    )
                                st = p1st.tile([128, 512], BF16, tag="kqst")
                                nc.vector.tensor_scalar_add(
                                    st[:], ps[:],
                                    bkq_sb[:, pi * HN + m:pi * HN + m + 1],
                                )
                                nc.sync.dma_start(
                                    out=out_d[m * 128:(m + 1) * 128,
                                              tq * 512:(tq + 1) * 512],
                                    in_=st[:],
                                )

                # V projection (natural layout), col halves of 512
                with (
                    tc.tile_pool(name="p1vst", bufs=3) as p1vst,
                    tc.tile_pool(name="ps_v", bufs=2, space="PSUM") as ps_v,
                ):
                    for cc in range(DHD // 512):
                        wvn3 = wvns[cc]
                        for tb in range(NB):
                            ps = ps_v.tile([128, 512], F32, tag="psv")
                            for k in range(DK):
                                tq, tr = tb // 4, tb % 4
                                nc.tensor.matmul(
                                    ps[:],
                                    slab4[:, tq, k,
                                          tr * 128:(tr + 1) * 128],
                                    wvn3[:, k],
                                    start=(k == 0),
                                    stop=(k == DK - 1),
                                )
                            st = p1vst.tile([128, 512], BF16, tag="vst")
                            nc.vector.tensor_tensor(
                                st[:], ps[:],
                                bv_sb[:, cc * 512:(cc + 1) * 512], ADD,
                            )
                            nc.sync.dma_start(
                                out=v_ds[cc][tb * 128:(tb + 1) * 128, :],
                                in_=st[:],
                            )

            # ---------------- phase 2: attention per head ----------------
            with (
                tc.tile_pool(name="aslab", bufs=1) as paslab,
                tc.tile_pool(name="p3w", bufs=1) as p3w,
                tc.tile_pool(name="p3b", bufs=1) as p3b,
            ):
                at_sb = paslab.tile([128, HN * T], BF16, tag="aslab")
                at3 = at_sb[:].rearrange("p (h t) -> p h t", h=HN)
                with (
                    tc.tile_pool(name="pmask", bufs=1) as pmask,
                    tc.tile_pool(name="ph", bufs=2) as ph,
                    tc.tile_pool(name="pe", bufs=6) as pe_pool,
                    tc.tile_pool(name="paraw", bufs=4) as paraw,
                    tc.tile_pool(name="psm", bufs=4) as psm,
                    tc.tile_pool(name="plb", bufs=4) as plb,
                    tc.tile_pool(name="ps_s", bufs=4, space="PSUM") as ps_s,
                    tc.tile_pool(name="ps_a", bufs=2, space="PSUM") as ps_a,
                    tc.tile_pool(name="ps_l", bufs=2, space="PSUM") as ps_l,
                ):
                    mask_sb = pmask.tile([128, 128], BF16, tag="mask")
                    nc.scalar.dma_start(out=mask_sb[:], in_=mask[:])
                    v_drs = [
                        v_d.rearrange("(jb p) c -> p jb c", p=128)
                        for v_d in v_ds
                    ]
                    for h in range(HN):
                        # head 0/1 loads ride the ACT queue (idle at the
                        # phase boundary; SP still drains V spills)
                        eng = nc.scalar if h < 2 else nc.sync
                        kt_h = ph.tile([128, T], BF16, tag="kth")
                        eng.dma_start(
                            out=kt_h[:], in_=kt_d[h * 128:(h + 1) * 128, :]
                        )
                        qt_h = ph.tile([128, T], BF16, tag="qth")
                        eng.dma_start(
                            out=qt_h[:], in_=qt_d[h * 128:(h + 1) * 128, :]
                        )
                        v_h = ph.tile([128, NB * 128], BF16, tag="vh")
                        v_h3 = v_h[:].rearrange("p (j c) -> p j c", j=NB)
                        hl = (h % 4) * 128
                        eng.dma_start(
                            out=v_h3,
                            in_=v_drs[h // 4][:, :, hl:hl + 128],
                        )
                        if h == 0:
                            # phase-3 weight prefetch, behind head-0 loads
                            bo_sb = p3b.tile([128, D], F32, tag="bo")
                            nc.scalar.dma_start(out=bo_sb[:], in_=bob[:])
                            wo_sb = p3w.tile([128, HN * D], BF16, tag="wo")
                            wo3 = wo_sb[:].rearrange("p (h n) -> p h n", h=HN)
                            nc.scalar.dma_start(out=wo_sb[:], in_=wo[:])

                        def finish_supertile(s, psa, psl):
                            # copy accumulators out fast (frees the PSUM
                            # banks), then the reciprocal/normalize chain
                            araw = paraw.tile([128, SS], F32, tag="araw")
                            nc.vector.tensor_copy(araw[:], psa[:])
                            l_sb = psm.tile([1, SS], F32, tag="lsb")
                            nc.vector.tensor_copy(l_sb[:], psl[:])
                            linv = psm.tile([1, SS], F32, tag="linv")
                            nc.vector.reciprocal_approx_fast(linv[:], l_sb[:])
                            lb = plb.tile([128, SS], F32, tag="lb")
                            nc.gpsimd.partition_broadcast(
                                lb[:], linv[:], channels=128
                            )
                            nc.vector.tensor_tensor(
                                at3[:, h, s * SS:(s + 1) * SS],
                                araw[:], lb[:], MULT,
                            )

                        from collections import deque
                        pending = deque()
                        for s in range(NST):
                            psa = ps_a.tile([128, SS], F32, tag="psa")
                            psl = ps_l.tile([1, SS], F32, tag="psl")
                            nj = JPS * s + JPS
                            for j in range(nj):
                                # diagonal key blocks only score the
                                # queries they can see
                                off = max(0, (j - JPS * s) * 128)
                                w = SS - off
                                pss = ps_s.tile([128, SS], F32, tag="pss")
                                nc.tensor.matmul(
                                    pss[:, :w],
                                    kt_h[:, j * 128:(j + 1) * 128],
                                    qt_h[:, s * SS + off:(s + 1) * SS],
                                    start=True, stop=True,
                                )
                                et = pe_pool.tile([128, SS], BF16, tag="et")
                                nc.scalar.activation(
                                    et[:, :w], pss[:, :w], EXP, scale=SCALE
                                )
                                if j >= JPS * s:
                                    nc.vector.tensor_mul(
                                        et[:, :128], et[:, :128], mask_sb[:]
                                    )
                                if len(pending) >= 2:
                                    pending.popleft()()
                                vj = v_h3[:, j, :]
                                first, last = (j == 0), (j == nj - 1)

                                def consume(et=et, vj=vj, first=first,
                                            last=last, psa=psa, psl=psl,
                                            s=s, off=off, w=w):
                                    nc.tensor.matmul(
                                        psa[:, off:off + w], vj, et[:, :w],
                                        start=first, stop=last,
                                    )
                                    nc.tensor.matmul(
                                        psl[:, off:off + w], ones_col[:],
                                        et[:, :w],
                                        start=first, stop=last,
                                    )
                                    if last:
                                        finish_supertile(s, psa, psl)

                                pending.append(consume)
                        while pending:
                            pending.popleft()()

                # ---------------- phase 3: output projection ----------------
                with (
                    tc.tile_pool(name="p3st", bufs=3) as p3st,
                    tc.tile_pool(name="ps_o", bufs=2, space="PSUM") as ps_o,
                ):
                    for tb in range(NB):
                        for cc in range(D // 512):
                            pso = ps_o.tile([128, 512], F32, tag="pso")
                            for hh in range(HN):
                                nc.tensor.matmul(
                                    pso[:],
                                    at3[:, hh, tb * 128:(tb + 1) * 128],
                                    wo3[:, hh, cc * 512:(cc + 1) * 512],
                                    start=(hh == 0),
                                    stop=(hh == HN - 1),
                                )
                            ost = p3st.tile([128, 512], F32, tag="ost")
                            nc.vector.tensor_tensor(
                                ost[:], pso[:],
                                bo_sb[:, cc * 512:(cc + 1) * 512], ADD,
                            )
                            nc.sync.dma_start(
                                out=o[tb * 128:(tb + 1) * 128,
                                      cc * 512:(cc + 1) * 512],
                                in_=ost[:],
                            )
    nc.compile()
    return nc


def _slabify(w, dk=16, p=128):
    """[D, N] -> [128, DK*N] with layout [p, k, n] (k = row chunk)."""
    d, n = w.shape
    assert d == dk * p
    return np.ascontiguousarray(
        w.reshape(dk, p, n).transpose(1, 0, 2).reshape(p, dk * n)
    )


def make_core_inputs(cfg, inputs):
    """Per-core input maps. Core index = 2*b + hg."""
    c = _derived(cfg)
    B, T, D, H = c["B"], c["T"], c["D"], c["H"]
    HN, DHD, DK = c["HN"], c["DHD"], c["DK"]
    f32 = np.float32
    bf16 = ml_dtypes.bfloat16
    x = np.asarray(inputs["x"], f32)
    Wk = np.asarray(inputs["Wk"], f32)
    Wq = np.asarray(inputs["Wq"], f32)
    Wv = np.asarray(inputs["Wv"], f32)
    Wo = np.asarray(inputs["Wo"], f32)
    bk = np.asarray(inputs["bk"], f32)
    bq = np.asarray(inputs["bq"], f32)
    bv = np.asarray(inputs["bv"], f32)
    bo = np.asarray(inputs["bo"], f32)

    p = np.arange(128)[:, None]
    cq = np.arange(128)[None, :]
    mask = (p <= cq).astype(bf16)
    ones_c = np.ones((128, 1), bf16)

    per_hg = []
    for hg in range(2):
        sl = slice(hg * DHD, (hg + 1) * DHD)
        bkq = np.empty((128, 2 * HN), f32)
        bkq[:, :HN] = bk[sl].reshape(HN, 128).T
        bkq[:, HN:] = bq[sl].reshape(HN, 128).T
        # K/Q walls: [p, proj, m, k, n128]
        wall = np.empty((128, 2, HN, DK, 128), f32)
        for pi, W in enumerate((Wk, Wq)):
            ws = W[:, sl].reshape(DK, 128, HN, 128)
            wall[:, pi] = ws.transpose(1, 2, 0, 3)
        # Wo slab: [p, h, n] with p = dh within head h
        wos = Wo[sl, :].reshape(HN, 128, D).transpose(1, 0, 2)
        per_hg.append({
            "wkq": np.ascontiguousarray(wall.reshape(128, -1)).astype(bf16),
            # [p, cc, k, n512] to match the device's (cc, k, n) split
            "wv": np.ascontiguousarray(
                Wv[:, sl].reshape(DK, 128, 2, 512).transpose(1, 2, 0, 3)
                .reshape(128, -1)).astype(bf16),
            "wo": np.ascontiguousarray(wos.reshape(128, -1)).astype(bf16),
            "bkq": bkq,
            "bvb": np.ascontiguousarray(
                np.broadcast_to(bv[sl], (128, DHD))),
            "bob": (np.ascontiguousarray(np.broadcast_to(bo, (128, D)))
                    if hg == 0 else np.zeros((128, D), f32)),
            "mask": mask,
            "ones_c": ones_c,
        })

    in_maps = []
    for b in range(B):
        # x^T slab, tq-major: [p, tq, k, 512]
        xt = x[b].T  # [D, T]
        xts = xt.reshape(DK, 128, T // 512, 512).transpose(1, 2, 0, 3)
        xt_pre = np.ascontiguousarray(xts.reshape(128, -1)).astype(bf16)
        for hg in range(2):
            in_maps.append({"xt": xt_pre, **per_hg[hg]})
    return in_maps


def run_cores(cfg, nc, in_maps, trace=False, tmpdir=None):
    c = _derived(cfg)
    n = c["N_CORES"]
    res = run_bass_kernel_spmd(
        nc, in_maps, list(range(n)), trace=trace, tmpdir=tmpdir
    )
    B, T, D = c["B"], c["T"], c["D"]
    out = np.empty((B, T, D), dtype=np.float32)
    for b in range(B):
        out[b] = res.results[2 * b]["o"]
        out[b] += res.results[2 * b + 1]["o"]
    return out, res


_NC_CACHE = {}


def kernel(x, Wq, bq, Wk, bk, Wv, bv, Wo, bo):
    cfg = PROD_CFG
    key = tuple(sorted(cfg.items()))
    if key not in _NC_CACHE:
        _NC_CACHE[key] = build_nc(cfg)
    nc = _NC_CACHE[key]
    inputs = dict(x=x, Wq=Wq, bq=bq, Wk=Wk, bk=bk, Wv=Wv, bv=bv, Wo=Wo, bo=bo)
    in_maps = make_core_inputs(cfg, inputs)
    out, _ = run_cores(cfg, nc, in_maps)
    return out


# revision 9
# speedup vs baseline: 1.9822x; 1.0375x over previous
"""Causal multi-head attention (B=4, T=2048, D=2048, H=16) on 8 Trainium2
NeuronCores via Bass/Tile, SPMD with zero collectives.

Sharding: core = (batch b, head-half hg). Each core owns one batch and 8 of
the 16 heads: it projects Q/K/V for its 1024-column slice of Wq/Wk/Wv over
the full sequence, runs causal attention for its 8 heads, and computes the
partial output projection A @ Wo[hg*1024:(hg+1)*1024, :]. The host feeds
x^T per batch and sums the two partials per batch (bo is folded into the
hg=0 partial on device via a broadcast tile).

All matmul operands are bf16 (cast on the host): bf16 runs the PE at 1
cycle/row like f32r but its LDWEIGHTS uses the fast weight load path (f32
cannot). Accumulation stays f32 in PSUM; softmax denominators accumulate
exactly via per-tile ones-vector matmuls into a dedicated PSUM bank.

The projection and attention phases are FUSED per head: V is projected
first (it is the only projection that needs a DRAM round trip, for the
per-head column gather), then for each head K^T and Q^T are projected
straight into SBUF tiles (no DRAM round trip) and immediately consumed by
that head's attention. Each head's softmax (ACT engine) overlaps the next
head's projection matmuls, so the tensor engine never waits on the scalar
engine. Wo streams in per-head chunks during the head loop.

The host pre-arranges x^T and every weight into the exact SBUF slab layout
([partition, chunk, col]) so every load is a contiguous full-bandwidth
DMA, split across the two hardware queues (SP + Activation).
"""
import numpy as np
import ml_dtypes

import concourse.bacc as bacc
import concourse.mybir as mybir
from concourse.tile import TileContext
from concourse.bass_utils import run_bass_kernel_spmd

F32 = mybir.dt.float32
BF16 = mybir.dt.bfloat16
EXP = mybir.ActivationFunctionType.Exp
MULT = mybir.AluOpType.mult
ADD = mybir.AluOpType.add

PROD_CFG = dict(B=4, T=2048, D=2048, H=16)


def _derived(cfg):
    B, T, D, H = cfg["B"], cfg["T"], cfg["D"], cfg["H"]
    d = dict(cfg)
    d.update(
        HN=H // 2,            # heads per core
        DHD=(H // 2) * (D // H),  # local head dim total (1024)
        DK=D // 128,          # contraction chunks of x^T
        SS=512,               # query supertile width
        DH=D // H,            # 128
        N_CORES=2 * B,
    )
    return d


def build_nc(cfg):
    c = _derived(cfg)
    T, D = c["T"], c["D"]
    HN, DHD, DK, SS = c["HN"], c["DHD"], c["DK"], c["SS"]
    NB = T // 128          # key blocks (16)
    NST = T // SS          # supertiles (4)
    JPS = SS // 128        # key blocks per supertile (4)
    NTQ = T // 512         # t chunks (4)
    SCALE = float(c["DH"] ** -0.5)

    nc = bacc.Bacc(
        "TRN2", target_bir_lowering=False, debug=False, num_devices=c["N_CORES"]
    )
    # host-prearranged slab layouts: [128, chunk*cols] contiguous
    xt = nc.dram_tensor("xt", [128, DK * T], BF16, kind="ExternalInput").ap()
    wkq = nc.dram_tensor(
        "wkq", [128, 2 * HN * DK * 128], BF16, kind="ExternalInput"
    ).ap()
    wv = nc.dram_tensor("wv", [128, DK * DHD], BF16, kind="ExternalInput").ap()
    wo = nc.dram_tensor("wo", [128, HN * D], BF16, kind="ExternalInput").ap()
    bkq = nc.dram_tensor("bkq", [128, 2 * HN], F32, kind="ExternalInput").ap()
    bvb = nc.dram_tensor("bvb", [128, DHD], F32, kind="ExternalInput").ap()
    bob = nc.dram_tensor("bob", [128, D], F32, kind="ExternalInput").ap()
    mask = nc.dram_tensor("mask", [128, 128], BF16, kind="ExternalInput").ap()
    ones_c_in = nc.dram_tensor("ones_c", [128, 1], BF16, kind="ExternalInput").ap()
    o = nc.dram_tensor("o", [T, D], F32, kind="ExternalOutput").ap()

    # per-half V scratch so head-0's reload isn't gated on the full tensor
    v_ds = [
        nc.dram_tensor(f"v_scratch{i}", [T, DHD // 2], BF16).ap()
        for i in range(2)
    ]

    with TileContext(nc) as tc:
        with tc.tile_pool(name="const", bufs=1) as pconst:
            ones_col = pconst.tile([128, 1], BF16, tag="ones_col")
            nc.scalar.dma_start(out=ones_col[:], in_=ones_c_in[:])
            bkq_sb = pconst.tile([128, 2 * HN], F32, tag="bkq")
            nc.scalar.dma_start(out=bkq_sb[:], in_=bkq[:])
            bv_sb = pconst.tile([128, DHD], F32, tag="bv")
            nc.scalar.dma_start(out=bv_sb[:], in_=bvb[:])

            with (
                tc.tile_pool(name="slab", bufs=1) as pslab,
                tc.tile_pool(name="aslab", bufs=1) as paslab,
                tc.tile_pool(name="p3w", bufs=1) as p3w,
                tc.tile_pool(name="p3b", bufs=1) as p3b,
                tc.tile_pool(name="p1w", bufs=2) as p1w,
            ):
                # x^T slab, tq-major [p, tq, k, 512]; the first chunk is
                # split across both DMA queues so compute starts early
                slab = pslab.tile([128, DK * T], BF16, tag="slab")
                slab4 = slab[:].rearrange(
                    "p (tq k t) -> p tq k t", tq=NTQ, k=DK
                )
                xt4 = xt.rearrange("p (tq k t) -> p tq k t", tq=NTQ, k=DK)
                nc.sync.dma_start(
                    out=slab4[:, 0, :DK // 2], in_=xt4[:, 0, :DK // 2]
                )
                nc.scalar.dma_start(
                    out=slab4[:, 0, DK // 2:], in_=xt4[:, 0, DK // 2:]
                )
                for tq in range(1, NTQ):
                    nc.sync.dma_start(out=slab4[:, tq], in_=xt4[:, tq])

                wkq4 = wkq.rearrange(
                    "p (i m r) -> p i m r", i=2, m=HN
                )

                def load_wchunk(pi, m):
                    wc = p1w.tile([128, DK * 128], BF16,
                                  tag=f"wc{pi}")
                    nc.scalar.dma_start(out=wc[:], in_=wkq4[:, pi, m])
                    return wc[:].rearrange("p (k n) -> p k n", k=DK)

                # ---------------- V projection (both halves) ----------------
                with (
                    tc.tile_pool(name="p1wv", bufs=2) as p1wv,
                    tc.tile_pool(name="p1vst", bufs=3) as p1vst,
                    tc.tile_pool(name="ps_v", bufs=2, space="PSUM") as ps_v,
                ):
                    wv3 = wv.rearrange("p (cc r) -> p cc r", cc=2)
                    wvns = []
                    for cc in range(DHD // 512):
                        wvn = p1wv.tile([128, DK * 512], BF16, tag="wvn")
                        wvns.append(wvn[:].rearrange("p (k n) -> p k n", k=DK))
                        nc.scalar.dma_start(out=wvn[:], in_=wv3[:, cc])
                    # K/Q chunks for head 0 arrive during V compute
                    wchunks = [load_wchunk(0, 0), load_wchunk(1, 0)]
                    for cc in range(DHD // 512):
                        wvn3 = wvns[cc]
                        for tb in range(NB):
                            ps = ps_v.tile([128, 512], F32, tag="psv")
                            for k in range(DK):
                                tq, tr = tb // 4, tb % 4
                                nc.tensor.matmul(
                                    ps[:],
                                    slab4[:, tq, k,
                                          tr * 128:(tr + 1) * 128],
                                    wvn3[:, k],
                                    start=(k == 0),
                                    stop=(k == DK - 1),
                                )
                            st = p1vst.tile([128, 512], BF16, tag="vst")
                            nc.vector.tensor_tensor(
                                st[:], ps[:],
                                bv_sb[:, cc * 512:(cc + 1) * 512], ADD,
                            )
                            nc.sync.dma_start(
                                out=v_ds[cc][tb * 128:(tb + 1) * 128, :],
                                in_=st[:],
                            )

                # -------- fused per-head K/Q projection + attention --------
                at_sb = paslab.tile([128, HN * T], BF16, tag="aslab")
                at3 = at_sb[:].rearrange("p (h t) -> p h t", h=HN)
                wo3 = None
                with (
                    tc.tile_pool(name="pmask", bufs=1) as pmask,
                    tc.tile_pool(name="ph", bufs=2) as ph,
                    tc.tile_pool(name="pe", bufs=6) as pe_pool,
                    tc.tile_pool(name="paraw", bufs=4) as paraw,
                    tc.tile_pool(name="psm", bufs=2) as psm,
                    tc.tile_pool(name="plb", bufs=2) as plb,
                    tc.tile_pool(name="ps_kq", bufs=2, space="PSUM") as ps_kq,
                    tc.tile_pool(name="ps_s", bufs=3, space="PSUM") as ps_s,
                    tc.tile_pool(name="ps_a", bufs=2, space="PSUM") as ps_a,
                    tc.tile_pool(name="ps_l", bufs=1, space="PSUM") as ps_l,
                ):
                    mask_sb = pmask.tile([128, 128], BF16, tag="mask")
                    nc.scalar.dma_start(out=mask_sb[:], in_=mask[:])
                    v_drs = [
                        v_d.rearrange("(jb p) c -> p jb c", p=128)
                        for v_d in v_ds
                    ]
                    wo_h = wo.rearrange("p (h n) -> p h n", h=HN)
                    for h in range(HN):
                        # prefetches for this head / the next
                        v_h = ph.tile([128, NB * 128], BF16, tag="vh")
                        v_h3 = v_h[:].rearrange("p (j c) -> p j c", j=NB)
                        hl = (h % 4) * 128
                        nc.sync.dma_start(
                            out=v_h3,
                            in_=v_drs[h // 4][:, :, hl:hl + 128],
                        )
                        if h == 0:
                            bo_sb = p3b.tile([128, D], F32, tag="bo")
                            nc.scalar.dma_start(out=bo_sb[:], in_=bob[:])
                            wo_sb = p3w.tile([128, HN * D], BF16, tag="wo")
                            wo3 = wo_sb[:].rearrange(
                                "p (h n) -> p h n", h=HN
                            )
                        # Wo streams one head-chunk per iteration
                        nc.scalar.dma_start(out=wo3[:, h], in_=wo_h[:, h])

                        # K^T and Q^T straight into SBUF
                        kt_h = ph.tile([128, T], BF16, tag="kth")
                        qt_h = ph.tile([128, T], BF16, tag="qth")
                        wck, wcq = wchunks
                        if h + 1 < HN:
                            wchunks = [load_wchunk(0, h + 1),
                                       load_wchunk(1, h + 1)]
                        for pi, (wc3, dst) in enumerate(
                                ((wck, kt_h), (wcq, qt_h))):
                            for tq in range(NTQ):
                                ps = ps_kq.tile([128, 512], F32, tag="pskq")
                                for k in range(DK):
                                    nc.tensor.matmul(
                                        ps[:],
                                        wc3[:, k],
                                        slab4[:, tq, k],
                                        start=(k == 0),
                                        stop=(k == DK - 1),
                                    )
                                nc.vector.tensor_scalar_add(
                                    dst[:, tq * 512:(tq + 1) * 512], ps[:],
                                    bkq_sb[:, pi * HN + h:pi * HN + h + 1],
                                )

                        def finish_supertile(s, psa, psl):
                            # copy accumulators out fast (frees the PSUM
                            # banks), then the reciprocal/normalize chain
                            araw = paraw.tile([128, SS], F32, tag="araw")
                            nc.vector.tensor_copy(araw[:], psa[:])
                            l_sb = psm.tile([1, SS], F32, tag="lsb")
                            nc.vector.tensor_copy(l_sb[:], psl[:])
                            linv = psm.tile([1, SS], F32, tag="linv")
                            nc.vector.reciprocal_approx_fast(linv[:], l_sb[:])
                            lb = plb.tile([128, SS], F32, tag="lb")
                            nc.gpsimd.partition_broadcast(
                                lb[:], linv[:], channels=128
                            )
                            nc.vector.tensor_tensor(
                                at3[:, h, s * SS:(s + 1) * SS],
                                araw[:], lb[:], MULT,
                            )

                        from collections import deque
                        pending = deque()
                        for s in range(NST):
                            psa = ps_a.tile([128, SS], F32, tag="psa")
                            psl = ps_l.tile([1, SS], F32, tag="psl")
                            nj = JPS * s + JPS
                            for j in range(nj):
                                # diagonal key blocks only score the
                                # queries they can see
                                off = max(0, (j - JPS * s) * 128)
                                w = SS - off
                                pss = ps_s.tile([128, SS], F32, tag="pss")
                                nc.tensor.matmul(
                                    pss[:, :w],
                                    kt_h[:, j * 128:(j + 1) * 128],
                                    qt_h[:, s * SS + off:(s + 1) * SS],
                                    start=True, stop=True,
                                )
                                et = pe_pool.tile([128, SS], BF16, tag="et")
                                nc.scalar.activation(
                                    et[:, :w], pss[:, :w], EXP, scale=SCALE
                                )
                                if j >= JPS * s:
                                    nc.vector.tensor_mul(
                                        et[:, :128], et[:, :128], mask_sb[:]
                                    )
                                if len(pending) >= 2:
                                    pending.popleft()()
                                vj = v_h3[:, j, :]
                                first, last = (j == 0), (j == nj - 1)

                                def consume(et=et, vj=vj, first=first,
                                            last=last, psa=psa, psl=psl,
                                            s=s, off=off, w=w):
                                    nc.tensor.matmul(
                                        psa[:, off:off + w], vj, et[:, :w],
                                        start=first, stop=last,
                                    )
                                    nc.tensor.matmul(
                                        psl[:, off:off + w], ones_col[:],
                                        et[:, :w],
                                        start=first, stop=last,
                                    )
                                    if last:
                                        finish_supertile(s, psa, psl)

                                pending.append(consume)
                        while pending:
                            pending.popleft()()

                # ---------------- output projection ----------------
                with (
                    tc.tile_pool(name="p3st", bufs=3) as p3st,
                    tc.tile_pool(name="ps_o", bufs=2, space="PSUM") as ps_o,
                ):
                    for tb in range(NB):
                        for cc in range(D // 512):
                            pso = ps_o.tile([128, 512], F32, tag="pso")
                            for hh in range(HN):
                                nc.tensor.matmul(
                                    pso[:],
                                    at3[:, hh, tb * 128:(tb + 1) * 128],
                                    wo3[:, hh, cc * 512:(cc + 1) * 512],
                                    start=(hh == 0),
                                    stop=(hh == HN - 1),
                                )
                            ost = p3st.tile([128, 512], F32, tag="ost")
                            nc.vector.tensor_tensor(
                                ost[:], pso[:],
                                bo_sb[:, cc * 512:(cc + 1) * 512], ADD,
                            )
                            nc.sync.dma_start(
                                out=o[tb * 128:(tb + 1) * 128,
                                      cc * 512:(cc + 1) * 512],
                                in_=ost[:],
                            )
    nc.compile()
    return nc


def make_core_inputs(cfg, inputs):
    """Per-core input maps. Core index = 2*b + hg."""
    c = _derived(cfg)
    B, T, D, H = c["B"], c["T"], c["D"], c["H"]
    HN, DHD, DK = c["HN"], c["DHD"], c["DK"]
    f32 = np.float32
    bf16 = ml_dtypes.bfloat16
    x = np.asarray(inputs["x"], f32)
    Wk = np.asarray(inputs["Wk"], f32)
    Wq = np.asarray(inputs["Wq"], f32)
    Wv = np.asarray(inputs["Wv"], f32)
    Wo = np.asarray(inputs["Wo"], f32)
    bk = np.asarray(inputs["bk"], f32)
    bq = np.asarray(inputs["bq"], f32)
    bv = np.asarray(inputs["bv"], f32)
    bo = np.asarray(inputs["bo"], f32)

    p = np.arange(128)[:, None]
    cq = np.arange(128)[None, :]
    mask = (p <= cq).astype(bf16)
    ones_c = np.ones((128, 1), bf16)

    per_hg = []
    for hg in range(2):
        sl = slice(hg * DHD, (hg + 1) * DHD)
        bkq = np.empty((128, 2 * HN), f32)
        bkq[:, :HN] = bk[sl].reshape(HN, 128).T
        bkq[:, HN:] = bq[sl].reshape(HN, 128).T
        # K/Q walls: [p, proj, m, k, n128]
        wall = np.empty((128, 2, HN, DK, 128), f32)
        for pi, W in enumerate((Wk, Wq)):
            ws = W[:, sl].reshape(DK, 128, HN, 128)
            wall[:, pi] = ws.transpose(1, 2, 0, 3)
        # Wo slab: [p, h, n] with p = dh within head h
        wos = Wo[sl, :].reshape(HN, 128, D).transpose(1, 0, 2)
        per_hg.append({
            "wkq": np.ascontiguousarray(wall.reshape(128, -1)).astype(bf16),
            # [p, cc, k, n512] to match the device's (cc, k, n) split
            "wv": np.ascontiguousarray(
                Wv[:, sl].reshape(DK, 128, 2, 512).transpose(1, 2, 0, 3)
                .reshape(128, -1)).astype(bf16),
            "wo": np.ascontiguousarray(wos.reshape(128, -1)).astype(bf16),
            "bkq": bkq,
            "bvb": np.ascontiguousarray(
                np.broadcast_to(bv[sl], (128, DHD))),
            "bob": (np.ascontiguousarray(np.broadcast_to(bo, (128, D)))
                    if hg == 0 else np.zeros((128, D), f32)),
            "mask": mask,
            "ones_c": ones_c,
        })

    in_maps = []
    for b in range(B):
        # x^T slab, tq-major: [p, tq, k, 512]
        xt = x[b].T  # [D, T]
        xts = xt.reshape(DK, 128, T // 512, 512).transpose(1, 2, 0, 3)
        xt_pre = np.ascontiguousarray(xts.reshape(128, -1)).astype(bf16)
        for hg in range(2):
            in_maps.append({"xt": xt_pre, **per_hg[hg]})
    return in_maps


def run_cores(cfg, nc, in_maps, trace=False, tmpdir=None):
    c = _derived(cfg)
    n = c["N_CORES"]
    res = run_bass_kernel_spmd(
        nc, in_maps, list(range(n)), trace=trace, tmpdir=tmpdir
    )
    B, T, D = c["B"], c["T"], c["D"]
    out = np.empty((B, T, D), dtype=np.float32)
    for b in range(B):
        out[b] = res.results[2 * b]["o"]
        out[b] += res.results[2 * b + 1]["o"]
    return out, res


_NC_CACHE = {}


def kernel(x, Wq, bq, Wk, bk, Wv, bv, Wo, bo):
    cfg = PROD_CFG
    key = tuple(sorted(cfg.items()))
    if key not in _NC_CACHE:
        _NC_CACHE[key] = build_nc(cfg)
    nc = _NC_CACHE[key]
    inputs = dict(x=x, Wq=Wq, bq=bq, Wk=Wk, bk=bk, Wv=Wv, bv=bv, Wo=Wo, bo=bo)
    in_maps = make_core_inputs(cfg, inputs)
    out, _ = run_cores(cfg, nc, in_maps)
    return out
